# revision 14
# baseline (speedup 1.0000x reference)
import os
import numpy as np

# ---- problem constants (hardcoded; kernel.py must be self-contained) ----
IMG, WS, SHIFT = 32, 8, 4
C, HEADS, DEPTH = 512, 16, 24
E_DIM, N_E, B = 256, 8192, 8
L = IMG * IMG            # 1024
NW = WS * WS             # 64 tokens per window
HD = C // HEADS          # 32
NWIN = (IMG // WS) ** 2  # 16
FH = 4 * C               # 2048
P = 128
VBLK = HD + 2            # 34 (32 vals + softmax-denominator col + pad)
VW = HEADS * VBLK        # 544
NT = L // P              # 8 token tiles
KC = C // P              # 4 k-tiles over C
KE = E_DIM // P          # 2 k-tiles over E_DIM
NH = FH // P             # 16 fc1 out-channel tiles
SCALE = HD ** -0.5

_DEPTH = int(os.environ.get("BT_DEPTH", DEPTH))
_NCORES = int(os.environ.get("BT_NCORES", 8))


# ---- host-side helpers (mirror reference.py) ----
def _rel_index():
    coords = np.stack(np.meshgrid(np.arange(WS), np.arange(WS), indexing='ij'))
    cf = coords.reshape(2, -1)
    rel = (cf[:, :, None] - cf[:, None, :]).transpose(1, 2, 0)
    rel[:, :, 0] += WS - 1
    rel[:, :, 1] += WS - 1
    rel[:, :, 0] *= 2 * WS - 1
    return rel.sum(-1)  # [NW, NW] int


def _shift_mask():
    img = np.zeros((IMG, IMG), np.float32)
    cnt = 0
    sl = (slice(0, -WS), slice(-WS, -SHIFT), slice(-SHIFT, None))
    for hs in sl:
        for ws_ in sl:
            img[hs, ws_] = cnt
            cnt += 1
    win = img.reshape(IMG // WS, WS, IMG // WS, WS).transpose(0, 2, 1, 3).reshape(-1, NW)
    diff = win[:, None, :] - win[:, :, None]
    return np.where(diff != 0, -100.0, 0.0).astype(np.float32)  # [NWIN, NW, NW]


def _win_perm():
    """raster token index -> window-major position; perm[t_raster] = t_dev"""
    t = np.arange(L).reshape(IMG, IMG)
    wm = t.reshape(IMG // WS, WS, IMG // WS, WS).transpose(0, 2, 1, 3).reshape(-1)
    inv = np.empty(L, np.int64)
    inv[wm] = np.arange(L)
    return wm, inv  # wm: dev->raster, inv: raster->dev


_WM, _WM_INV = _win_perm()
_REL = _rel_index()
_MASK = _shift_mask()


def _prepare(inputs):
    import ml_dtypes
    bf16 = ml_dtypes.bfloat16
    f32 = lambda a: np.ascontiguousarray(a, dtype=np.float32)
    b16 = lambda a: np.ascontiguousarray(np.asarray(a, np.float32), dtype=bf16)
    x = np.asarray(inputs['x'], np.float32)          # [B, L, E]
    dec_w = np.asarray(inputs['dec_w'], np.float32)  # [C, E]
    dec_b = np.asarray(inputs['dec_b'], np.float32)
    pos = np.asarray(inputs['pos_embed'], np.float32)[0]  # [L, C]
    n1w = np.asarray(inputs['n1w'], np.float32)
    n1b = np.asarray(inputs['n1b'], np.float32)
    qkv_w = np.asarray(inputs['qkv_w'], np.float32)
    qkv_b = np.asarray(inputs['qkv_b'], np.float32)
    proj_w = np.asarray(inputs['proj_w'], np.float32)
    proj_b = np.asarray(inputs['proj_b'], np.float32)
    rel_bias = np.asarray(inputs['rel_bias'], np.float32)
    n2w = np.asarray(inputs['n2w'], np.float32)
    n2b = np.asarray(inputs['n2b'], np.float32)
    fc1_w = np.asarray(inputs['fc1_w'], np.float32)
    fc1_b = np.asarray(inputs['fc1_b'], np.float32)
    fc2_w = np.asarray(inputs['fc2_w'], np.float32)
    fc2_b = np.asarray(inputs['fc2_b'], np.float32)
    normf_w = np.asarray(inputs['normf_w'], np.float32)
    normf_b = np.asarray(inputs['normf_b'], np.float32)
    pred_w = np.asarray(inputs['pred_w'], np.float32)
    pred_b = np.asarray(inputs['pred_b'], np.float32)

    D = _DEPTH
    sh = {}
    sh['decw'] = b16(dec_w.T)                                    # [E, C]
    sh['posb'] = f32((pos + dec_b[None, :])[_WM])                # [L, C]

    wqk = np.empty((D, C, 2 * C), np.float32)
    bqk = np.zeros((D, P, 8), np.float32)
    wvp = np.zeros((D, C, VW), np.float32)
    vb = np.zeros((D, P, VW), np.float32)
    ab = np.empty((D, NT, 4, P, 4, NW), np.float32)  # (wp, rg, 2w*tk, hi, tq)
    wp_ = np.empty((D, C, C), np.float32)
    pb = np.empty((D, P, C), np.float32)
    f1 = np.empty((D, C, FH), np.float32)
    f1b = np.empty((D, P, NH), np.float32)
    f2 = np.empty((D, FH, C), np.float32)
    f2b = np.empty((D, P, C), np.float32)

    for i in range(D):
        Wm = qkv_w[i] * n1w[i][None, :]           # [3C, C]
        bm = qkv_w[i] @ n1b[i] + qkv_b[i]         # [3C]
        Wm = Wm.copy()
        bm = bm.copy()
        Wm[:C] *= SCALE
        bm[:C] *= SCALE
        wqk[i] = Wm[:2 * C].T                     # [C, 2C]
        bqk[i] = bm[:2 * C].reshape(8, P).T       # bias for out-channel tile mo at [:, mo]
        # v with padded 34-blocks; ones column for the softmax denominator
        for h in range(HEADS):
            wvp[i][:, h * VBLK:h * VBLK + HD] = Wm[2 * C + h * HD:2 * C + (h + 1) * HD].T
            vb[i][:, h * VBLK:h * VBLK + HD] = bm[2 * C + h * HD:2 * C + (h + 1) * HD][None, :]
            vb[i][:, h * VBLK + HD] = 1.0
        # attention additive bias [tk, tq] per (win, head)
        bias = rel_bias[i][_REL]                  # [tq, tk, HEADS]
        shift = (i % 2) == 1
        for w in range(NWIN):
            for h in range(HEADS):
                a = bias[:, :, h].T               # [tk, tq]
                if shift:
                    a = a + _MASK[w].T
                wp2, w01 = w // 2, w % 2
                rg, hi = h % 4, h // 4
                ab[i, wp2, rg, w01 * NW:(w01 + 1) * NW, hi, :] = a
        wp_[i] = proj_w[i].T
        pb[i] = np.broadcast_to(proj_b[i][None, :], (P, C))
        f1[i] = (fc1_w[i] * n2w[i][None, :]).T    # [C, FH]
        f1b[i] = (fc1_w[i] @ n2b[i] + fc1_b[i]).reshape(NH, P).T
        f2[i] = fc2_w[i].T
        f2b[i] = np.broadcast_to(fc2_b[i][None, :], (P, C))

    sh['wqk'] = b16(wqk)
    sh['bqk'] = f32(bqk)
    sh['wvp'] = b16(wvp)
    sh['vb'] = f32(vb)
    # ab partition-major for one contiguous DMA per layer: [D, P, NT, 4, 4*NW]
    sh['ab'] = np.ascontiguousarray(
        ab.reshape(D, NT, 4, P, 4 * NW).transpose(0, 3, 1, 2, 4).astype(bf16))
    sh['wp'] = b16(wp_)
    sh['pb'] = f32(pb)
    sh['f1'] = b16(f1)
    sh['f1b'] = f32(f1b)
    sh['f2'] = b16(f2)
    sh['f2b'] = f32(f2b)
    sh['nfw'] = f32(normf_w.reshape(KC, P).T)     # [P, KC]
    sh['nfb'] = f32(normf_b.reshape(KC, P).T)
    sh['pw'] = b16(pred_w.T)                      # [C, N_E]
    sh['pwb'] = f32(pred_b.reshape(N_E // P, P).T)  # [P, 64]
    # per-core xT in device token order: [E, L] bf16
    xts = [np.ascontiguousarray(x[c][_WM].T.astype(bf16)) for c in range(B)]
    return sh, xts


# ---- device program ----
_BUILD_CACHE = {}


def _build():
    key = (_DEPTH,)
    if key in _BUILD_CACHE:
        return _BUILD_CACHE[key]
    import concourse.bass as bass
    import concourse.mybir as mybir
    import concourse.tile as tile
    from concourse import bacc
    from concourse.masks import make_identity
    from contextlib import ExitStack

    F32 = mybir.dt.float32
    BF16 = mybir.dt.bfloat16
    AF = mybir.ActivationFunctionType
    ALU = mybir.AluOpType
    AX = mybir.AxisListType
    D = _DEPTH

    nc = bacc.Bacc("TRN2", target_bir_lowering=False, debug=False, num_devices=_NCORES)

    dr = {}
    def din(name, shape, dt):
        dr[name] = nc.dram_tensor(name, list(shape), dt, kind="ExternalInput").ap()
    din('xT', (E_DIM, L), BF16)
    din('decw', (E_DIM, C), BF16)
    din('posb', (L, C), F32)
    din('wqk', (D, C, 2 * C), BF16)
    din('bqk', (D, P, 8), F32)
    din('wvp', (D, C, VW), BF16)
    din('vb', (D, P, VW), F32)
    din('ab', (D, P, NT, 4, 4 * NW), BF16)
    din('wp', (D, C, C), BF16)
    din('pb', (D, P, C), F32)
    din('f1', (D, C, FH), BF16)
    din('f1b', (D, P, NH), F32)
    din('f2', (D, FH, C), BF16)
    din('f2b', (D, P, C), F32)
    din('nfw', (P, KC), F32)
    din('nfb', (P, KC), F32)
    din('pw', (C, N_E), BF16)
    din('pwb', (P, N_E // P), F32)
    outT = nc.dram_tensor("outT", [N_E, L], F32, kind="ExternalOutput").ap()

    with tile.TileContext(nc) as tc, ExitStack() as ES:
        # ---------- persistent SBUF pools ----------
        cst = ES.enter_context(tc.tile_pool(name="cst", bufs=1))
        ident_f = cst.tile([P, P], F32)
        make_identity(nc, ident_f)
        ident_b = cst.tile([P, P], BF16)
        nc.scalar.copy(ident_b[:], ident_f[:])

        xp = ES.enter_context(tc.tile_pool(name="xp", bufs=1))
        hp = ES.enter_context(tc.tile_pool(name="hp", bufs=1))
        hTp = ES.enter_context(tc.tile_pool(name="hTp", bufs=2))
        qkp = ES.enter_context(tc.tile_pool(name="qkp", bufs=1))
        vp = ES.enter_context(tc.tile_pool(name="vp", bufs=1))
        attp = ES.enter_context(tc.tile_pool(name="attp", bufs=1))
        ppool = ES.enter_context(tc.tile_pool(name="ppool", bufs=2))
        stp = ES.enter_context(tc.tile_pool(name="stp", bufs=2))
        recp = ES.enter_context(tc.tile_pool(name="recp", bufs=2))
        gp = ES.enter_context(tc.tile_pool(name="gp", bufs=3))
        bp = ES.enter_context(tc.tile_pool(name="bp", bufs=2))

        x = xp.tile([P, NT, C], F32)

        # ---------- layer-norm (stats per tile, rsqrt batched per 4-tile group;
        # rsqrt on DVE via bit-trick + 2 Newton iters: no ACT Sqrt table swaps) ----------
        UINT32 = mybir.dt.uint32

        def ln_stats(mvg, tt, j):
            st6 = stp.tile([P, 6], F32, name="st6", tag=f"st6{tt % 2}")
            nc.vector.bn_stats(st6[:], x[:, tt])
            nc.vector.bn_aggr(mvg[:, j], st6[:])

        def ln_group(mvg, tag):
            rstd = stp.tile([P, 4], F32, name="rstd", tag=f"rstd{tag}")
            nb = stp.tile([P, 4], F32, name="nb", tag=f"nb{tag}")
            ve = stp.tile([P, 4], F32, name="ve", tag=f"ve{tag}")
            nc.vector.tensor_scalar_add(ve[:], mvg[:, :, 1:2], 1e-5)
            nc.scalar.activation(rstd[:], ve[:], AF.Sqrt)
            nc.vector.reciprocal(rstd[:], rstd[:])
            nc.vector.tensor_mul(nb[:], mvg[:, :, 0:1], rstd[:])
            nc.vector.tensor_scalar_mul(nb[:], nb[:], -1.0)
            return rstd, nb

        def ln_apply(dst, tt, rstd, nb, j):
            nc.scalar.activation(dst[:, tt], x[:, tt], AF.Identity,
                                 bias=nb[:, j:j + 1], scale=rstd[:, j:j + 1])

        # transpose token-major [P, NT, C] bf16 -> C-major [P, KC, L] bf16
        # tt-outer; after each odd tile, cb(sa) can emit the shift-permute
        # pieces whose source tokens are now complete (overlaps PE with DVE/ACT)
        def transpose_to(hT, src, tpool, cb=None):
            for tt in range(NT):
                for ct in range(KC):
                    tps = tpool.tile([P, P], BF16, name="tp")
                    nc.tensor.transpose(tps[:], src[:, tt, ct * P:(ct + 1) * P], ident_b[:])
                    if (ct + tt) % 2 == 0:
                        nc.scalar.copy(hT[:, ct, tt * P:(tt + 1) * P], tps[:])
                    else:
                        nc.vector.tensor_copy(hT[:, ct, tt * P:(tt + 1) * P], tps[:])
                if cb is not None and tt % 2 == 1:
                    cb(tt // 2)

        # shift permute between A-order (unshifted window-major) and B-order.
        # Emits only the pieces reading source row-block `src_a` (= token tiles
        # 2*src_a, 2*src_a+1), spread across gpsimd/vector/scalar.
        def permute_pieces(dstT, srcT, fwd, src_a):
            G = IMG // WS  # 4
            sv = srcT[:].rearrange("p k (a b i j) -> p k a b i j", a=G, b=G, i=WS, j=WS)
            dv = dstT[:].rearrange("p k (a b i j) -> p k a b i j", a=G, b=G, i=WS, j=WS)
            engs = (nc.gpsimd.tensor_copy, nc.vector.tensor_copy, nc.scalar.copy)
            idx = src_a
            for qa in range(2):
                for qb in range(2):
                    di = slice(0, 4) if qa == 0 else slice(4, 8)
                    si = slice(4, 8) if qa == 0 else slice(0, 4)
                    dj = slice(0, 4) if qb == 0 else slice(4, 8)
                    sj = slice(4, 8) if qb == 0 else slice(0, 4)
                    for a in range(G):
                        sa = (a + qa) % G
                        if (sa if fwd else a) != src_a:
                            continue
                        if qb == 0:
                            bpairs = [(slice(0, G), slice(0, G))]
                        else:
                            bpairs = [(slice(0, G - 1), slice(1, G)), (slice(G - 1, G), slice(0, 1))]
                        for db, sb_ in bpairs:
                            for ct in range(KC):
                                eng = engs[idx % 3]
                                idx += 1
                                if fwd:
                                    eng(dv[:, ct, a, db, di, dj],
                                        sv[:, ct, sa, sb_, si, sj])
                                else:
                                    eng(dv[:, ct, sa, sb_, si, sj],
                                        sv[:, ct, a, db, di, dj])

        # ---------- dec ----------
        with tc.tile_pool(name="decp", bufs=1) as decp, \
             tc.tile_pool(name="dps", bufs=2, space="PSUM") as dps:
            xT_sb = decp.tile([P, KE, L], BF16)
            nc.sync.dma_start(xT_sb[:], dr['xT'].rearrange("(k p) t -> p k t", p=P))
            decw_sb = decp.tile([P, KE, C], BF16)
            nc.sync.dma_start(decw_sb[:], dr['decw'].rearrange("(k p) c -> p k c", p=P))
            for tt in range(NT):
                pos_t = decp.tile([P, C], F32, name="pos_t", tag="pos", bufs=2)
                nc.sync.dma_start(pos_t[:], dr['posb'][tt * P:(tt + 1) * P, :])
                ps = dps.tile([P, C], F32)
                for kk in range(KE):
                    nc.tensor.matmul(ps[:], xT_sb[:, kk, tt * P:(tt + 1) * P],
                                     decw_sb[:, kk, :], start=(kk == 0), stop=(kk == KE - 1))
                nc.vector.tensor_add(x[:, tt], ps[:], pos_t[:])

        # LN1 + hT for layer 0 (layer 0 is unshifted)
        h = hp.tile([P, NT, C], BF16, name="h")
        for g2 in range(2):
            mvg = stp.tile([P, 4, 2], F32, name="mvg", tag=f"mvgp{g2}")
            for j in range(4):
                ln_stats(mvg, g2 * 4 + j, j)
            rstd0, nb0 = ln_group(mvg, f"p{g2}")
            for j in range(4):
                ln_apply(h, g2 * 4 + j, rstd0, nb0, j)
        with tc.tile_pool(name="tp0", bufs=4, space="PSUM") as tpool:
            hT = hTp.tile([P, KC, L], BF16, name="hT")
            transpose_to(hT, h, tpool)

        # ---------- weight pools (freed before the pred phase) ----------
        with tc.tile_pool(name="wqkp", bufs=2) as wqkp, \
             tc.tile_pool(name="wvpp", bufs=2) as wvpp, \
             tc.tile_pool(name="wpp", bufs=2) as wpp, \
             tc.tile_pool(name="f1p", bufs=1) as f1p, \
             tc.tile_pool(name="f2p", bufs=1) as f2p, \
             tc.tile_pool(name="abp", bufs=1) as abp:

            # ---------- layers ----------
            for i in range(D):
                shift = (i % 2) == 1
                shift_next = ((i + 1) % 2) == 1
                # whole-layer weight DMAs (one dispatch per tensor)
                wqk_sb = wqkp.tile([P, KC, 2 * C], BF16, name="wqk")
                nc.sync.dma_start(wqk_sb[:], dr['wqk'][i].rearrange("(k p) m -> p k m", p=P))
                bqk_sb = bp.tile([P, 8], F32, name="bqk", tag="bqk")
                nc.sync.dma_start(bqk_sb[:], dr['bqk'][i])
                wvp_sb = wvpp.tile([P, KC, VW], BF16, name="wvp")
                nc.sync.dma_start(wvp_sb[:], dr['wvp'][i].rearrange("(k p) m -> p k m", p=P))
                vb_sb = bp.tile([P, VW], F32, name="vb", tag="vb")
                nc.sync.dma_start(vb_sb[:], dr['vb'][i])
                ab_sb = abp.tile([P, NT, 4, 4 * NW], BF16, name="ab")
                nc.sync.dma_start(ab_sb[:], dr['ab'][i])
                wp_sb = wpp.tile([P, KC, C], BF16, name="wp")
                nc.sync.dma_start(wp_sb[:], dr['wp'][i].rearrange("(k p) m -> p k m", p=P))
                pb_sb = bp.tile([P, C], F32, name="pb", tag="pb")
                nc.sync.dma_start(pb_sb[:], dr['pb'][i])
                f1_sb = f1p.tile([P, KC, FH], BF16, name="f1")
                nc.sync.dma_start(f1_sb[:], dr['f1'][i].rearrange("(k p) m -> p k m", p=P))
                f1b_sb = bp.tile([P, NH], F32, name="f1b", tag="f1b")
                nc.sync.dma_start(f1b_sb[:], dr['f1b'][i])
                f2_sb = f2p.tile([P, NH, C], BF16, name="f2")
                nc.sync.dma_start(f2_sb[:], dr['f2'][i].rearrange("(h p) c -> p h c", p=P))
                f2b_sb = bp.tile([P, C], F32, name="f2b", tag="f2b")
                nc.sync.dma_start(f2b_sb[:], dr['f2b'][i])

                # ---- qk ---- (hT was prepared, incl. fwd shift-permute, at the
                # end of the previous layer)
                qkT = qkp.tile([P, 8, L], BF16, name="qkT")
                with tc.tile_pool(name="mmps1", bufs=2, space="PSUM") as mmps, \
                     tc.tile_pool(name="vps", bufs=2, space="PSUM") as vps:
                    for mo in range(8):
                        for tc2 in range(2):
                            ps = mmps.tile([P, C], F32, name="mm")
                            for kk in range(KC):
                                nc.tensor.matmul(ps[:], wqk_sb[:, kk, mo * P:(mo + 1) * P],
                                                 hT[:, kk, tc2 * 512:(tc2 + 1) * 512],
                                                 start=(kk == 0), stop=(kk == KC - 1))
                            nc.vector.tensor_scalar_add(
                                qkT[:, mo, tc2 * 512:(tc2 + 1) * 512], ps[:],
                                bqk_sb[:, mo:mo + 1])
                    # ---- v ---- (pairs of tiles: all N=512 MMs first, then the
                    # N=32 tails back-to-back so their LDWEIGHTS don't collide
                    # with a 512-wide stream)
                    v_aug = vp.tile([P, NT, VW], BF16, name="vaug")
                    for tp2_ in range(NT // 2):
                        pspair = []
                        for half in range(2):
                            tt = tp2_ * 2 + half
                            ps = vps.tile([P, VW], F32, name="vps")
                            pspair.append(ps)
                            for kk in range(KC):
                                nc.tensor.matmul(ps[:, 0:512], hT[:, kk, tt * P:(tt + 1) * P],
                                                 wvp_sb[:, kk, 0:512], start=(kk == 0),
                                                 stop=(kk == KC - 1), skip_group_check=True)
                        for half in range(2):
                            tt = tp2_ * 2 + half
                            ps = pspair[half]
                            for kk in range(KC):
                                nc.tensor.matmul(ps[:, 512:VW], hT[:, kk, tt * P:(tt + 1) * P],
                                                 wvp_sb[:, kk, 512:VW], start=(kk == 0),
                                                 stop=(kk == KC - 1), skip_group_check=True)
                            nc.vector.tensor_add(v_aug[:, tt], ps[:], vb_sb[:])

                # pre-add proj bias into the residual (gpsimd; overlaps attention)
                for tt in range(NT):
                    nc.gpsimd.tensor_add(x[:, tt], x[:, tt], pb_sb[:])

                # ---- attention: S + softmax + AV ----
                att = attp.tile([P, NT, C], BF16, name="att")
                with tc.tile_pool(name="sps", bufs=1, space="PSUM") as sps, \
                     tc.tile_pool(name="avps", bufs=1, space="PSUM") as avps:
                    for wp2 in range(NT):
                        pts = []
                        for rg in range(4):
                            sp = sps.tile([P, 4, NW], F32, name=f"s{rg}", tag=f"s{rg}")
                            spf = sp[:].rearrange("p a b -> p (a b)")
                            nc.tensor.matmul(spf, ident_b[:], ab_sb[:, wp2, rg],
                                             start=True, stop=False, skip_group_check=True)
                            for hi in range(4):
                                for w01 in range(2):
                                    qs = qkT[rg * HD:(rg + 1) * HD, hi,
                                             (wp2 * 2 + w01) * NW:(wp2 * 2 + w01 + 1) * NW]
                                    ks = qkT[rg * HD:(rg + 1) * HD, 4 + hi,
                                             (wp2 * 2 + w01) * NW:(wp2 * 2 + w01 + 1) * NW]
                                    nc.tensor.matmul(sp[w01 * NW:(w01 + 1) * NW, hi, :], ks, qs,
                                                     start=False,
                                                     stop=(hi == 3 and w01 == 1),
                                                     tile_position=(rg * HD, w01 * NW),
                                                     skip_group_check=True)
                            pt = ppool.tile([P, 4, NW], BF16, name=f"p{rg}", tag=f"p{rg}")
                            nc.scalar.activation(pt[:].rearrange("p a b -> p (a b)"), spf, AF.Exp)
                            pts.append(pt)
                        for w01 in range(2):
                            rows = slice(w01 * NW, (w01 + 1) * NW)
                            rec = recp.tile([P, HEADS], F32, name=f"rec{w01}", tag=f"rec{w01}")
                            for half in range(2):
                                av = avps.tile([P, 8, VBLK], F32, name=f"av{w01}{half}", tag=f"av{w01}{half}")
                                for hh in range(8):
                                    hglob = half * 8 + hh
                                    hi, rg = hglob // 4, hglob % 4
                                    nc.tensor.matmul(
                                        av[rows, hh, :], pts[rg][rows, hi, :],
                                        v_aug[rows, wp2, hglob * VBLK:(hglob + 1) * VBLK],
                                        start=True, stop=True,
                                        tile_position=(w01 * NW, w01 * NW))
                                nc.vector.reciprocal(rec[rows, half * 8:(half + 1) * 8],
                                                     av[rows, :, HD])
                                rb = rec[rows, half * 8:(half + 1) * 8] \
                                    .rearrange("p (a b) -> p a b", b=1).to_broadcast((NW, 8, HD))
                                dst = att[rows, wp2, half * 256:(half + 1) * 256] \
                                    .rearrange("p (a b) -> p a b", b=HD)
                                nc.vector.tensor_mul(dst, av[rows, :, 0:HD], rb)

                # ---- attn transpose back (+ inverse shift permute, fused) ----
                with tc.tile_pool(name="tp2", bufs=4, space="PSUM") as tpool:
                    aT_B = hTp.tile([P, KC, L], BF16, name="hT")
                    if shift:
                        aT = hTp.tile([P, KC, L], BF16, name="hT")
                        transpose_to(aT_B, att, tpool,
                                     cb=lambda sa: permute_pieces(aT, aT_B, False, sa))
                    else:
                        transpose_to(aT_B, att, tpool)
                        aT = aT_B

                # ---- proj + residual + LN2 (grouped per 4 tiles) ----
                h2 = hp.tile([P, NT, C], BF16, name="h")
                with tc.tile_pool(name="mmps2", bufs=2, space="PSUM") as mmps:
                    for g2 in range(2):
                        mvg = stp.tile([P, 4, 2], F32, name="mvg", tag=f"mvgb{g2}")
                        for j in range(4):
                            tt = g2 * 4 + j
                            ps = mmps.tile([P, C], F32, name="mm")
                            for kk in range(KC):
                                nc.tensor.matmul(ps[:], aT[:, kk, tt * P:(tt + 1) * P],
                                                 wp_sb[:, kk, :], start=(kk == 0), stop=(kk == KC - 1))
                            nc.vector.tensor_add(x[:, tt], ps[:], x[:, tt])
                            ln_stats(mvg, tt, j)
                        rstd2, nb2 = ln_group(mvg, f"b{g2}")
                        for j in range(4):
                            tt = g2 * 4 + j
                            ln_apply(h2, tt, rstd2, nb2, j)
                            # pre-add fc2 bias (gpsimd; after LN2 read x)
                            nc.gpsimd.tensor_add(x[:, tt], x[:, tt], f2b_sb[:])
                with tc.tile_pool(name="tp3", bufs=4, space="PSUM") as tpool:
                    h2T = hTp.tile([P, KC, L], BF16, name="hT")
                    transpose_to(h2T, h2, tpool)

                # ---- MLP (epilogue emits LN1 of next layer / final LN) ----
                h = hp.tile([P, NT, C], BF16, name="h")
                with tc.tile_pool(name="mmps3", bufs=2, space="PSUM") as mmps, \
                     tc.tile_pool(name="fc2ps", bufs=1, space="PSUM") as fc2ps:
                    for tc2 in range(2):
                        pso = [fc2ps.tile([P, C], F32, name=f"fc2_{j}", tag=f"fc2_{j}") for j in range(4)]
                        for ho in range(NH):
                            ps1 = mmps.tile([P, C], F32, name="mm")
                            for kk in range(KC):
                                nc.tensor.matmul(ps1[:], f1_sb[:, kk, ho * P:(ho + 1) * P],
                                                 h2T[:, kk, tc2 * 512:(tc2 + 1) * 512],
                                                 start=(kk == 0), stop=(kk == KC - 1))
                            g = gp.tile([P, C], BF16, name="g")
                            nc.scalar.activation(g[:], ps1[:], AF.Gelu, bias=f1b_sb[:, ho:ho + 1])
                            for j in range(4):
                                nc.tensor.matmul(pso[j][:], g[:, j * P:(j + 1) * P],
                                                 f2_sb[:, ho, :],
                                                 start=(ho == 0), stop=(ho == NH - 1))
                        mvg = stp.tile([P, 4, 2], F32, name="mvg", tag=f"mvga{tc2}")
                        for j in range(4):
                            tt = tc2 * 4 + j
                            nc.vector.tensor_add(x[:, tt], pso[j][:], x[:, tt])
                            ln_stats(mvg, tt, j)
                        rstd1, nb1 = ln_group(mvg, f"a{tc2}")
                        for j in range(4):
                            ln_apply(h, tc2 * 4 + j, rstd1, nb1, j)
                # hT for next layer (or hf for the final head), incl. fwd permute
                with tc.tile_pool(name="tp4", bufs=4, space="PSUM") as tpool:
                    if i < D - 1:
                        hT_A = hTp.tile([P, KC, L], BF16, name="hT")
                        if shift_next:
                            hTn = hTp.tile([P, KC, L], BF16, name="hT")
                            transpose_to(hT_A, h, tpool,
                                         cb=lambda sa: permute_pieces(hTn, hT_A, True, sa))
                            hT = hTn
                        else:
                            transpose_to(hT_A, h, tpool)
                            hT = hT_A
                    # final layer: h holds plain-LN output for the pred head

        # ---------- final gelu(LN)*nfw+nfb transpose + pred ----------
        nfw_sb = bp.tile([P, KC], F32, name="nfw", tag="nfw")
        nc.sync.dma_start(nfw_sb[:], dr['nfw'])
        nfb_sb = bp.tile([P, KC], F32, name="nfb", tag="nfb")
        nc.sync.dma_start(nfb_sb[:], dr['nfb'])
        pwb_sb = bp.tile([P, N_E // P], F32, name="pwb", tag="pwb", bufs=1)
        nc.sync.dma_start(pwb_sb[:], dr['pwb'])
        with tc.tile_pool(name="tpf", bufs=4, space="PSUM") as tpool, \
             tc.tile_pool(name="mmpsf", bufs=4, space="PSUM") as mmps, \
             tc.tile_pool(name="pwp", bufs=2) as pwp, \
             tc.tile_pool(name="outp", bufs=2) as outp:
            gT = hTp.tile([P, KC, L], BF16, name="hT")
            for ct in range(KC):
                for tt in range(NT):
                    tps = tpool.tile([P, P], BF16, name="tp")
                    nc.tensor.transpose(tps[:], h[:, tt, ct * P:(ct + 1) * P], ident_b[:])
                    nc.scalar.activation(gT[:, ct, tt * P:(tt + 1) * P], tps[:], AF.Gelu,
                                         bias=nfb_sb[:, ct:ct + 1], scale=nfw_sb[:, ct:ct + 1])
            # pred: out C-major [N_E, L]; 8 weight chunks, big batched output DMAs
            for ch in range(8):
                pwc = pwp.tile([P, KC, 8 * P], BF16, name="pwc")
                nc.sync.dma_start(pwc[:], dr['pw'][:, ch * 1024:(ch + 1) * 1024]
                                  .rearrange("(k p) m -> p k m", p=P))
                for tc2 in range(2):
                    osb = outp.tile([P, 8, 512], F32, name="osb")
                    for no8 in range(8):
                        ps = mmps.tile([P, 512], F32, name="mm")
                        for kk in range(KC):
                            nc.tensor.matmul(ps[:], pwc[:, kk, no8 * P:(no8 + 1) * P],
                                             gT[:, kk, tc2 * 512:(tc2 + 1) * 512],
                                             start=(kk == 0), stop=(kk == KC - 1))
                        no = ch * 8 + no8
                        nc.scalar.activation(osb[:, no8], ps[:], AF.Identity,
                                             bias=pwb_sb[:, no:no + 1])
                    nc.sync.dma_start(
                        outT[ch * 1024:(ch + 1) * 1024, tc2 * 512:(tc2 + 1) * 512]
                        .rearrange("(n p) l -> p n l", p=P), osb[:])

    nc.compile()
    _BUILD_CACHE[key] = nc
    return nc


LAST_RESULTS = None


def kernel(**inputs):
    global LAST_RESULTS
    from concourse import bass_utils
    sh, xts = _prepare(inputs)
    nc = _build()
    in_maps = []
    for c in range(_NCORES):
        m = dict(sh)
        m['xT'] = xts[c % B]
        in_maps.append(m)
    trace = os.environ.get("BT_TRACE", "0") == "1"
    if trace:
        try:
            import antenv.axon_hooks  # noqa: F401
        except ImportError:
            trace = False
    res = bass_utils.run_bass_kernel_spmd(nc, in_maps, core_ids=list(range(_NCORES)),
                                          trace=trace)
    LAST_RESULTS = res
    outs = []
    for c in range(B):
        oT = res.results[c % _NCORES]['outT']  # [N_E, L] in device token order
        o = oT.T[_WM_INV]                      # [L, N_E] raster order
        outs.append(o)
    return np.stack(outs).astype(np.float32)


# revision 20
# speedup vs baseline: 1.0001x; 1.0001x over previous
import os
import numpy as np

# ---- problem constants (hardcoded; kernel.py must be self-contained) ----
IMG, WS, SHIFT = 32, 8, 4
C, HEADS, DEPTH = 512, 16, 24
E_DIM, N_E, B = 256, 8192, 8
L = IMG * IMG            # 1024
NW = WS * WS             # 64 tokens per window
HD = C // HEADS          # 32
NWIN = (IMG // WS) ** 2  # 16
FH = 4 * C               # 2048
P = 128
VBLK = HD + 2            # 34 (32 vals + softmax-denominator col + pad)
VW = HEADS * VBLK        # 544
NT = L // P              # 8 token tiles
KC = C // P              # 4 k-tiles over C
KE = E_DIM // P          # 2 k-tiles over E_DIM
NH = FH // P             # 16 fc1 out-channel tiles
SCALE = HD ** -0.5

_DEPTH = int(os.environ.get("BT_DEPTH", DEPTH))
_NCORES = int(os.environ.get("BT_NCORES", 8))


# ---- host-side helpers (mirror reference.py) ----
def _rel_index():
    coords = np.stack(np.meshgrid(np.arange(WS), np.arange(WS), indexing='ij'))
    cf = coords.reshape(2, -1)
    rel = (cf[:, :, None] - cf[:, None, :]).transpose(1, 2, 0)
    rel[:, :, 0] += WS - 1
    rel[:, :, 1] += WS - 1
    rel[:, :, 0] *= 2 * WS - 1
    return rel.sum(-1)  # [NW, NW] int


def _shift_mask():
    img = np.zeros((IMG, IMG), np.float32)
    cnt = 0
    sl = (slice(0, -WS), slice(-WS, -SHIFT), slice(-SHIFT, None))
    for hs in sl:
        for ws_ in sl:
            img[hs, ws_] = cnt
            cnt += 1
    win = img.reshape(IMG // WS, WS, IMG // WS, WS).transpose(0, 2, 1, 3).reshape(-1, NW)
    diff = win[:, None, :] - win[:, :, None]
    return np.where(diff != 0, -100.0, 0.0).astype(np.float32)  # [NWIN, NW, NW]


def _win_perm():
    """raster token index -> window-major position; perm[t_raster] = t_dev"""
    t = np.arange(L).reshape(IMG, IMG)
    wm = t.reshape(IMG // WS, WS, IMG // WS, WS).transpose(0, 2, 1, 3).reshape(-1)
    inv = np.empty(L, np.int64)
    inv[wm] = np.arange(L)
    return wm, inv  # wm: dev->raster, inv: raster->dev


_WM, _WM_INV = _win_perm()
_REL = _rel_index()
_MASK = _shift_mask()


def _prepare(inputs):
    import ml_dtypes
    bf16 = ml_dtypes.bfloat16
    f32 = lambda a: np.ascontiguousarray(a, dtype=np.float32)
    b16 = lambda a: np.ascontiguousarray(np.asarray(a, np.float32), dtype=bf16)
    x = np.asarray(inputs['x'], np.float32)          # [B, L, E]
    dec_w = np.asarray(inputs['dec_w'], np.float32)  # [C, E]
    dec_b = np.asarray(inputs['dec_b'], np.float32)
    pos = np.asarray(inputs['pos_embed'], np.float32)[0]  # [L, C]
    n1w = np.asarray(inputs['n1w'], np.float32)
    n1b = np.asarray(inputs['n1b'], np.float32)
    qkv_w = np.asarray(inputs['qkv_w'], np.float32)
    qkv_b = np.asarray(inputs['qkv_b'], np.float32)
    proj_w = np.asarray(inputs['proj_w'], np.float32)
    proj_b = np.asarray(inputs['proj_b'], np.float32)
    rel_bias = np.asarray(inputs['rel_bias'], np.float32)
    n2w = np.asarray(inputs['n2w'], np.float32)
    n2b = np.asarray(inputs['n2b'], np.float32)
    fc1_w = np.asarray(inputs['fc1_w'], np.float32)
    fc1_b = np.asarray(inputs['fc1_b'], np.float32)
    fc2_w = np.asarray(inputs['fc2_w'], np.float32)
    fc2_b = np.asarray(inputs['fc2_b'], np.float32)
    normf_w = np.asarray(inputs['normf_w'], np.float32)
    normf_b = np.asarray(inputs['normf_b'], np.float32)
    pred_w = np.asarray(inputs['pred_w'], np.float32)
    pred_b = np.asarray(inputs['pred_b'], np.float32)

    D = _DEPTH
    sh = {}
    sh['decw'] = b16(dec_w.T)                                    # [E, C]
    sh['posb'] = f32((pos + dec_b[None, :])[_WM])                # [L, C]

    wqk = np.empty((D, C, 2 * C), np.float32)
    bqk = np.zeros((D, P, 8), np.float32)
    wvp = np.zeros((D, C, VW), np.float32)
    vb = np.zeros((D, P, VW), np.float32)
    ab = np.empty((D, NT, 4, P, 4, NW), np.float32)  # (wp, rg, 2w*tk, hi, tq)
    wp_ = np.empty((D, C, C), np.float32)
    pb = np.empty((D, P, C), np.float32)
    f1 = np.empty((D, C, FH), np.float32)
    f1b = np.empty((D, P, NH), np.float32)
    f2 = np.empty((D, FH, C), np.float32)
    f2b = np.empty((D, P, C), np.float32)

    for i in range(D):
        Wm = qkv_w[i] * n1w[i][None, :]           # [3C, C]
        bm = qkv_w[i] @ n1b[i] + qkv_b[i]         # [3C]
        Wm = Wm.copy()
        bm = bm.copy()
        Wm[:C] *= SCALE
        bm[:C] *= SCALE
        wqk[i] = Wm[:2 * C].T                     # [C, 2C]
        bqk[i] = bm[:2 * C].reshape(8, P).T       # bias for out-channel tile mo at [:, mo]
        # v with padded 34-blocks; ones column for the softmax denominator
        for h in range(HEADS):
            wvp[i][:, h * VBLK:h * VBLK + HD] = Wm[2 * C + h * HD:2 * C + (h + 1) * HD].T
            vb[i][:, h * VBLK:h * VBLK + HD] = bm[2 * C + h * HD:2 * C + (h + 1) * HD][None, :]
            vb[i][:, h * VBLK + HD] = 1.0
        # attention additive bias [tk, tq] per (win, head)
        bias = rel_bias[i][_REL]                  # [tq, tk, HEADS]
        shift = (i % 2) == 1
        for w in range(NWIN):
            for h in range(HEADS):
                a = bias[:, :, h].T               # [tk, tq]
                if shift:
                    a = a + _MASK[w].T
                wp2, w01 = w // 2, w % 2
                rg, hi = h % 4, h // 4
                ab[i, wp2, rg, w01 * NW:(w01 + 1) * NW, hi, :] = a
        wp_[i] = proj_w[i].T
        pb[i] = np.broadcast_to(proj_b[i][None, :], (P, C))
        f1[i] = (fc1_w[i] * n2w[i][None, :]).T    # [C, FH]
        f1b[i] = (fc1_w[i] @ n2b[i] + fc1_b[i]).reshape(NH, P).T
        f2[i] = fc2_w[i].T
        f2b[i] = np.broadcast_to(fc2_b[i][None, :], (P, C))

    sh['wqk'] = b16(wqk)
    sh['bqk'] = f32(bqk)
    sh['wvp'] = b16(wvp)
    sh['vb'] = f32(vb)
    # ab partition-major for one contiguous DMA per layer: [D, P, NT, 4, 4*NW]
    sh['ab'] = np.ascontiguousarray(
        ab.reshape(D, NT, 4, P, 4 * NW).transpose(0, 3, 1, 2, 4).astype(bf16))
    sh['wp'] = b16(wp_)
    sh['pb'] = f32(pb)
    sh['f1'] = b16(f1)
    sh['f1b'] = f32(f1b)
    sh['f2'] = b16(f2)
    sh['f2b'] = f32(f2b)
    sh['nfw'] = f32(normf_w.reshape(KC, P).T)     # [P, KC]
    sh['nfb'] = f32(normf_b.reshape(KC, P).T)
    sh['pw'] = b16(pred_w.T)                      # [C, N_E]
    sh['pwb'] = f32(pred_b.reshape(N_E // P, P).T)  # [P, 64]
    # per-core xT in device token order: [E, L] bf16
    xts = [np.ascontiguousarray(x[c][_WM].T.astype(bf16)) for c in range(B)]
    return sh, xts


# ---- device program ----
_BUILD_CACHE = {}


def _build():
    key = (_DEPTH,)
    if key in _BUILD_CACHE:
        return _BUILD_CACHE[key]
    import concourse.bass as bass
    import concourse.mybir as mybir
    import concourse.tile as tile
    from concourse import bacc
    from concourse.masks import make_identity
    from contextlib import ExitStack

    F32 = mybir.dt.float32
    BF16 = mybir.dt.bfloat16
    AF = mybir.ActivationFunctionType
    ALU = mybir.AluOpType
    AX = mybir.AxisListType
    D = _DEPTH

    nc = bacc.Bacc("TRN2", target_bir_lowering=False, debug=False, num_devices=_NCORES)

    dr = {}
    def din(name, shape, dt):
        dr[name] = nc.dram_tensor(name, list(shape), dt, kind="ExternalInput").ap()
    din('xT', (E_DIM, L), BF16)
    din('decw', (E_DIM, C), BF16)
    din('posb', (L, C), F32)
    din('wqk', (D, C, 2 * C), BF16)
    din('bqk', (D, P, 8), F32)
    din('wvp', (D, C, VW), BF16)
    din('vb', (D, P, VW), F32)
    din('ab', (D, P, NT, 4, 4 * NW), BF16)
    din('wp', (D, C, C), BF16)
    din('pb', (D, P, C), F32)
    din('f1', (D, C, FH), BF16)
    din('f1b', (D, P, NH), F32)
    din('f2', (D, FH, C), BF16)
    din('f2b', (D, P, C), F32)
    din('nfw', (P, KC), F32)
    din('nfb', (P, KC), F32)
    din('pw', (C, N_E), BF16)
    din('pwb', (P, N_E // P), F32)
    outT = nc.dram_tensor("outT", [N_E, L], F32, kind="ExternalOutput").ap()

    with tile.TileContext(nc) as tc, ExitStack() as ES:
        # ---------- persistent SBUF pools ----------
        cst = ES.enter_context(tc.tile_pool(name="cst", bufs=1))
        ident_f = cst.tile([P, P], F32)
        make_identity(nc, ident_f)
        ident_b = cst.tile([P, P], BF16)
        nc.scalar.copy(ident_b[:], ident_f[:])

        xp = ES.enter_context(tc.tile_pool(name="xp", bufs=1))
        hp = ES.enter_context(tc.tile_pool(name="hp", bufs=1))
        hTp = ES.enter_context(tc.tile_pool(name="hTp", bufs=2))
        qkp = ES.enter_context(tc.tile_pool(name="qkp", bufs=1))
        vp = ES.enter_context(tc.tile_pool(name="vp", bufs=1))
        attp = ES.enter_context(tc.tile_pool(name="attp", bufs=1))
        ppool = ES.enter_context(tc.tile_pool(name="ppool", bufs=2))
        stp = ES.enter_context(tc.tile_pool(name="stp", bufs=2))
        recp = ES.enter_context(tc.tile_pool(name="recp", bufs=2))
        gp = ES.enter_context(tc.tile_pool(name="gp", bufs=3))
        bp = ES.enter_context(tc.tile_pool(name="bp", bufs=2))

        x = xp.tile([P, NT, C], F32)

        # ---------- layer-norm (stats per tile, rsqrt batched per 4-tile group;
        # rsqrt on DVE via bit-trick + 2 Newton iters: no ACT Sqrt table swaps) ----------
        UINT32 = mybir.dt.uint32

        def ln_stats(mvg, tt, j):
            st6 = stp.tile([P, 6], F32, name="st6", tag=f"st6{tt % 2}")
            nc.vector.bn_stats(st6[:], x[:, tt])
            nc.vector.bn_aggr(mvg[:, j], st6[:])

        def emit_ln(dst, tt, tag):
            st6 = stp.tile([P, 6], F32, name="st6", tag=f"st6{tag}{tt % 4}")
            nc.vector.bn_stats(st6[:], x[:, tt])
            mv = stp.tile([P, 2], F32, name="mv", tag=f"mv{tag}{tt % 4}")
            nc.vector.bn_aggr(mv[:], st6[:])
            ve = stp.tile([P, 1], F32, name="ve", tag=f"ve{tag}{tt % 4}")
            nc.vector.tensor_scalar_add(ve[:], mv[:, 1:2], 1e-5)
            rstd = stp.tile([P, 1], F32, name="rstd", tag=f"rstd{tag}{tt % 4}")
            nc.scalar.activation(rstd[:], ve[:], AF.Sqrt)
            nc.vector.reciprocal(rstd[:], rstd[:])
            nb = stp.tile([P, 1], F32, name="nb", tag=f"nb{tag}{tt % 4}")
            nc.vector.tensor_mul(nb[:], mv[:, 0:1], rstd[:])
            nc.vector.tensor_scalar_mul(nb[:], nb[:], -1.0)
            nc.scalar.activation(dst[:, tt], x[:, tt], AF.Identity,
                                 bias=nb[:], scale=rstd[:])

        # transpose token-major [P, NT, C] bf16 -> C-major [P, KC, L] bf16
        # tt-outer; after each odd tile, cb(sa) can emit the shift-permute
        # pieces whose source tokens are now complete (overlaps PE with DVE/ACT)
        def transpose_to(hT, src, tpool, cb=None):
            for tt in range(NT):
                for ct in range(KC):
                    tps = tpool.tile([P, P], BF16, name="tp")
                    nc.tensor.transpose(tps[:], src[:, tt, ct * P:(ct + 1) * P], ident_b[:])
                    if (ct + tt) % 2 == 0:
                        nc.scalar.copy(hT[:, ct, tt * P:(tt + 1) * P], tps[:])
                    else:
                        nc.vector.tensor_copy(hT[:, ct, tt * P:(tt + 1) * P], tps[:])
                if cb is not None and tt % 2 == 1:
                    cb(tt // 2)

        # shift permute between A-order (unshifted window-major) and B-order.
        # Emits only the pieces reading source row-block `src_a` (= token tiles
        # 2*src_a, 2*src_a+1), spread across gpsimd/vector/scalar.
        def permute_pieces(dstT, srcT, fwd, src_a):
            G = IMG // WS  # 4
            sv = srcT[:].rearrange("p k (a b i j) -> p k a b i j", a=G, b=G, i=WS, j=WS)
            dv = dstT[:].rearrange("p k (a b i j) -> p k a b i j", a=G, b=G, i=WS, j=WS)
            engs = (nc.gpsimd.tensor_copy, nc.vector.tensor_copy, nc.scalar.copy)
            idx = src_a
            for qa in range(2):
                for qb in range(2):
                    di = slice(0, 4) if qa == 0 else slice(4, 8)
                    si = slice(4, 8) if qa == 0 else slice(0, 4)
                    dj = slice(0, 4) if qb == 0 else slice(4, 8)
                    sj = slice(4, 8) if qb == 0 else slice(0, 4)
                    for a in range(G):
                        sa = (a + qa) % G
                        if (sa if fwd else a) != src_a:
                            continue
                        if qb == 0:
                            bpairs = [(slice(0, G), slice(0, G))]
                        else:
                            bpairs = [(slice(0, G - 1), slice(1, G)), (slice(G - 1, G), slice(0, 1))]
                        for db, sb_ in bpairs:
                            for ct in range(KC):
                                eng = engs[idx % 3]
                                idx += 1
                                if fwd:
                                    eng(dv[:, ct, a, db, di, dj],
                                        sv[:, ct, sa, sb_, si, sj])
                                else:
                                    eng(dv[:, ct, sa, sb_, si, sj],
                                        sv[:, ct, a, db, di, dj])

        # ---------- dec ----------
        with tc.tile_pool(name="decp", bufs=1) as decp, \
             tc.tile_pool(name="dps", bufs=2, space="PSUM") as dps:
            xT_sb = decp.tile([P, KE, L], BF16)
            nc.sync.dma_start(xT_sb[:], dr['xT'].rearrange("(k p) t -> p k t", p=P))
            decw_sb = decp.tile([P, KE, C], BF16)
            nc.sync.dma_start(decw_sb[:], dr['decw'].rearrange("(k p) c -> p k c", p=P))
            for tt in range(NT):
                pos_t = decp.tile([P, C], F32, name="pos_t", tag="pos", bufs=2)
                nc.sync.dma_start(pos_t[:], dr['posb'][tt * P:(tt + 1) * P, :])
                ps = dps.tile([P, C], F32)
                for kk in range(KE):
                    nc.tensor.matmul(ps[:], xT_sb[:, kk, tt * P:(tt + 1) * P],
                                     decw_sb[:, kk, :], start=(kk == 0), stop=(kk == KE - 1))
                nc.vector.tensor_add(x[:, tt], ps[:], pos_t[:])

        # LN1 + hT for layer 0 (layer 0 is unshifted)
        h = hp.tile([P, NT, C], BF16, name="h")
        for tt in range(NT):
            emit_ln(h, tt, "a")
        with tc.tile_pool(name="tp0", bufs=4, space="PSUM") as tpool:
            hT = hTp.tile([P, KC, L], BF16, name="hT")
            transpose_to(hT, h, tpool)

        # ---------- weight pools (freed before the pred phase) ----------
        with tc.tile_pool(name="wqkp", bufs=2) as wqkp, \
             tc.tile_pool(name="wvpp", bufs=2) as wvpp, \
             tc.tile_pool(name="wpp", bufs=2) as wpp, \
             tc.tile_pool(name="f1p", bufs=1) as f1p, \
             tc.tile_pool(name="f2p", bufs=1) as f2p, \
             tc.tile_pool(name="abp", bufs=1) as abp:

            # ---------- layers ----------
            for i in range(D):
                shift = (i % 2) == 1
                shift_next = ((i + 1) % 2) == 1
                # whole-layer weight DMAs (one dispatch per tensor)
                wqk_sb = wqkp.tile([P, KC, 2 * C], BF16, name="wqk")
                nc.sync.dma_start(wqk_sb[:], dr['wqk'][i].rearrange("(k p) m -> p k m", p=P))
                bqk_sb = bp.tile([P, 8], F32, name="bqk", tag="bqk")
                nc.sync.dma_start(bqk_sb[:], dr['bqk'][i])
                wvp_sb = wvpp.tile([P, KC, VW], BF16, name="wvp")
                nc.sync.dma_start(wvp_sb[:], dr['wvp'][i].rearrange("(k p) m -> p k m", p=P))
                vb_sb = bp.tile([P, VW], F32, name="vb", tag="vb")
                nc.sync.dma_start(vb_sb[:], dr['vb'][i])
                ab_sb = abp.tile([P, NT, 4, 4 * NW], BF16, name="ab")
                nc.sync.dma_start(ab_sb[:], dr['ab'][i])
                wp_sb = wpp.tile([P, KC, C], BF16, name="wp")
                nc.sync.dma_start(wp_sb[:], dr['wp'][i].rearrange("(k p) m -> p k m", p=P))
                pb_sb = bp.tile([P, C], F32, name="pb", tag="pb")
                nc.sync.dma_start(pb_sb[:], dr['pb'][i])
                f1_sb = f1p.tile([P, KC, FH], BF16, name="f1")
                nc.sync.dma_start(f1_sb[:], dr['f1'][i].rearrange("(k p) m -> p k m", p=P))
                f1b_sb = bp.tile([P, NH], F32, name="f1b", tag="f1b")
                nc.sync.dma_start(f1b_sb[:], dr['f1b'][i])
                f2_sb = f2p.tile([P, NH, C], BF16, name="f2")
                nc.sync.dma_start(f2_sb[:], dr['f2'][i].rearrange("(h p) c -> p h c", p=P))
                f2b_sb = bp.tile([P, C], F32, name="f2b", tag="f2b")
                nc.sync.dma_start(f2b_sb[:], dr['f2b'][i])

                # ---- qk ---- (hT was prepared, incl. fwd shift-permute, at the
                # end of the previous layer)
                qkT = qkp.tile([P, 8, L], BF16, name="qkT")
                with tc.tile_pool(name="mmps1", bufs=2, space="PSUM") as mmps, \
                     tc.tile_pool(name="vps", bufs=2, space="PSUM") as vps:
                    for mo in range(8):
                        for tc2 in range(2):
                            ps = mmps.tile([P, C], F32, name="mm")
                            for kk in range(KC):
                                nc.tensor.matmul(ps[:], wqk_sb[:, kk, mo * P:(mo + 1) * P],
                                                 hT[:, kk, tc2 * 512:(tc2 + 1) * 512],
                                                 start=(kk == 0), stop=(kk == KC - 1))
                            nc.vector.tensor_scalar_add(
                                qkT[:, mo, tc2 * 512:(tc2 + 1) * 512], ps[:],
                                bqk_sb[:, mo:mo + 1])
                    # ---- v ---- (pairs of tiles: all N=512 MMs first, then the
                    # N=32 tails back-to-back so their LDWEIGHTS don't collide
                    # with a 512-wide stream)
                    v_aug = vp.tile([P, NT, VW], BF16, name="vaug")
                    for tp2_ in range(NT // 2):
                        pspair = []
                        for half in range(2):
                            tt = tp2_ * 2 + half
                            ps = vps.tile([P, VW], F32, name="vps")
                            pspair.append(ps)
                            for kk in range(KC):
                                nc.tensor.matmul(ps[:, 0:512], hT[:, kk, tt * P:(tt + 1) * P],
                                                 wvp_sb[:, kk, 0:512], start=(kk == 0),
                                                 stop=(kk == KC - 1), skip_group_check=True)
                        for half in range(2):
                            tt = tp2_ * 2 + half
                            ps = pspair[half]
                            for kk in range(KC):
                                nc.tensor.matmul(ps[:, 512:VW], hT[:, kk, tt * P:(tt + 1) * P],
                                                 wvp_sb[:, kk, 512:VW], start=(kk == 0),
                                                 stop=(kk == KC - 1), skip_group_check=True)
                            nc.vector.tensor_add(v_aug[:, tt], ps[:], vb_sb[:])

                # pre-add proj bias into the residual (gpsimd; overlaps attention)
                for tt in range(NT):
                    nc.gpsimd.tensor_add(x[:, tt], x[:, tt], pb_sb[:])

                # ---- attention: S + softmax + AV, software-pipelined so the
                # PE never waits on the exp latency (S of wp2+1 issues before
                # AV of wp2) ----
                att = attp.tile([P, NT, C], BF16, name="att")
                with tc.tile_pool(name="sps", bufs=1, space="PSUM") as sps, \
                     tc.tile_pool(name="avps", bufs=1, space="PSUM") as avps:
                    def emit_S(wp2):
                        pts = []
                        for rg in range(4):
                            sp = sps.tile([P, 4, NW], F32, name=f"s{rg}", tag=f"s{rg}")
                            spf = sp[:].rearrange("p a b -> p (a b)")
                            nc.tensor.matmul(spf, ident_b[:], ab_sb[:, wp2, rg],
                                             start=True, stop=False, skip_group_check=True)
                            for hi in range(4):
                                for w01 in range(2):
                                    qs = qkT[rg * HD:(rg + 1) * HD, hi,
                                             (wp2 * 2 + w01) * NW:(wp2 * 2 + w01 + 1) * NW]
                                    ks = qkT[rg * HD:(rg + 1) * HD, 4 + hi,
                                             (wp2 * 2 + w01) * NW:(wp2 * 2 + w01 + 1) * NW]
                                    nc.tensor.matmul(sp[w01 * NW:(w01 + 1) * NW, hi, :], ks, qs,
                                                     start=False,
                                                     stop=(hi == 3 and w01 == 1),
                                                     tile_position=(rg * HD, w01 * NW),
                                                     skip_group_check=True)
                            pt = ppool.tile([P, 4, NW], BF16, name=f"p{rg}", tag=f"p{rg}")
                            nc.scalar.activation(pt[:].rearrange("p a b -> p (a b)"), spf, AF.Exp)
                            pts.append(pt)
                        return pts

                    def emit_AV(wp2, pts):
                        for w01 in range(2):
                            rows = slice(w01 * NW, (w01 + 1) * NW)
                            rec = recp.tile([P, HEADS], F32, name=f"rec{w01}", tag=f"rec{w01}")
                            for half in range(2):
                                av = avps.tile([P, 8, VBLK], F32, name=f"av{w01}{half}", tag=f"av{w01}{half}")
                                for hh in range(8):
                                    hglob = half * 8 + hh
                                    hi, rg = hglob // 4, hglob % 4
                                    nc.tensor.matmul(
                                        av[rows, hh, :], pts[rg][rows, hi, :],
                                        v_aug[rows, wp2, hglob * VBLK:(hglob + 1) * VBLK],
                                        start=True, stop=True,
                                        tile_position=(w01 * NW, w01 * NW))
                                nc.vector.reciprocal(rec[rows, half * 8:(half + 1) * 8],
                                                     av[rows, :, HD])
                                rb = rec[rows, half * 8:(half + 1) * 8] \
                                    .rearrange("p (a b) -> p a b", b=1).to_broadcast((NW, 8, HD))
                                dst = att[rows, wp2, half * 256:(half + 1) * 256] \
                                    .rearrange("p (a b) -> p a b", b=HD)
                                nc.vector.tensor_mul(dst, av[rows, :, 0:HD], rb)

                    pts_prev = emit_S(0)
                    for wp2 in range(1, NT):
                        pts_cur = emit_S(wp2)
                        emit_AV(wp2 - 1, pts_prev)
                        pts_prev = pts_cur
                    emit_AV(NT - 1, pts_prev)

                # ---- attn transpose back (+ inverse shift permute, fused) ----
                with tc.tile_pool(name="tp2", bufs=4, space="PSUM") as tpool:
                    aT_B = hTp.tile([P, KC, L], BF16, name="hT")
                    if shift:
                        aT = hTp.tile([P, KC, L], BF16, name="hT")
                        transpose_to(aT_B, att, tpool,
                                     cb=lambda sa: permute_pieces(aT, aT_B, False, sa))
                    else:
                        transpose_to(aT_B, att, tpool)
                        aT = aT_B

                # ---- proj + residual + LN2 (grouped per 4 tiles) ----
                h2 = hp.tile([P, NT, C], BF16, name="h")
                with tc.tile_pool(name="mmps2", bufs=2, space="PSUM") as mmps:
                    for tt in range(NT):
                        ps = mmps.tile([P, C], F32, name="mm")
                        for kk in range(KC):
                            nc.tensor.matmul(ps[:], aT[:, kk, tt * P:(tt + 1) * P],
                                             wp_sb[:, kk, :], start=(kk == 0), stop=(kk == KC - 1))
                        nc.vector.tensor_add(x[:, tt], ps[:], x[:, tt])
                        emit_ln(h2, tt, "b")
                        # pre-add fc2 bias (gpsimd; after LN2 read x)
                        nc.gpsimd.tensor_add(x[:, tt], x[:, tt], f2b_sb[:])
                with tc.tile_pool(name="tp3", bufs=4, space="PSUM") as tpool:
                    h2T = hTp.tile([P, KC, L], BF16, name="hT")
                    transpose_to(h2T, h2, tpool)

                # ---- MLP (epilogue emits LN1 of next layer / final LN) ----
                h = hp.tile([P, NT, C], BF16, name="h")
                with tc.tile_pool(name="mmps3", bufs=2, space="PSUM") as mmps, \
                     tc.tile_pool(name="fc2ps", bufs=1, space="PSUM") as fc2ps:
                    for tc2 in range(2):
                        pso = [fc2ps.tile([P, C], F32, name=f"fc2_{j}", tag=f"fc2_{j}") for j in range(4)]

                        def emit_fc2(ho, g, last):
                            for j in range(4):
                                nc.tensor.matmul(pso[j][:], g[:, j * P:(j + 1) * P],
                                                 f2_sb[:, ho, :],
                                                 start=(ho == 0), stop=last)
                        g_prev = None
                        for ho in range(NH):
                            ps1 = mmps.tile([P, C], F32, name="mm")
                            for kk in range(KC):
                                nc.tensor.matmul(ps1[:], f1_sb[:, kk, ho * P:(ho + 1) * P],
                                                 h2T[:, kk, tc2 * 512:(tc2 + 1) * 512],
                                                 start=(kk == 0), stop=(kk == KC - 1))
                            g = gp.tile([P, C], BF16, name="g")
                            nc.scalar.activation(g[:], ps1[:], AF.Gelu, bias=f1b_sb[:, ho:ho + 1])
                            if g_prev is not None:
                                emit_fc2(ho - 1, g_prev, False)
                            g_prev = g
                        emit_fc2(NH - 1, g_prev, True)
                        for j in range(4):
                            tt = tc2 * 4 + j
                            nc.vector.tensor_add(x[:, tt], pso[j][:], x[:, tt])
                            emit_ln(h, tt, "a")
                # hT for next layer (or hf for the final head), incl. fwd permute
                with tc.tile_pool(name="tp4", bufs=4, space="PSUM") as tpool:
                    if i < D - 1:
                        hT_A = hTp.tile([P, KC, L], BF16, name="hT")
                        if shift_next:
                            hTn = hTp.tile([P, KC, L], BF16, name="hT")
                            transpose_to(hT_A, h, tpool,
                                         cb=lambda sa: permute_pieces(hTn, hT_A, True, sa))
                            hT = hTn
                        else:
                            transpose_to(hT_A, h, tpool)
                            hT = hT_A
                    # final layer: h holds plain-LN output for the pred head

        # ---------- final gelu(LN)*nfw+nfb transpose + pred ----------
        nfw_sb = bp.tile([P, KC], F32, name="nfw", tag="nfw")
        nc.sync.dma_start(nfw_sb[:], dr['nfw'])
        nfb_sb = bp.tile([P, KC], F32, name="nfb", tag="nfb")
        nc.sync.dma_start(nfb_sb[:], dr['nfb'])
        pwb_sb = bp.tile([P, N_E // P], F32, name="pwb", tag="pwb", bufs=1)
        nc.sync.dma_start(pwb_sb[:], dr['pwb'])
        with tc.tile_pool(name="tpf", bufs=4, space="PSUM") as tpool, \
             tc.tile_pool(name="mmpsf", bufs=4, space="PSUM") as mmps, \
             tc.tile_pool(name="pwp", bufs=2) as pwp, \
             tc.tile_pool(name="outp", bufs=2) as outp:
            gT = hTp.tile([P, KC, L], BF16, name="hT")
            for ct in range(KC):
                for tt in range(NT):
                    tps = tpool.tile([P, P], BF16, name="tp")
                    nc.tensor.transpose(tps[:], h[:, tt, ct * P:(ct + 1) * P], ident_b[:])
                    nc.scalar.activation(gT[:, ct, tt * P:(tt + 1) * P], tps[:], AF.Gelu,
                                         bias=nfb_sb[:, ct:ct + 1], scale=nfw_sb[:, ct:ct + 1])
            # pred: out C-major [N_E, L]; 8 weight chunks, big batched output DMAs
            for ch in range(8):
                pwc = pwp.tile([P, KC, 8 * P], BF16, name="pwc")
                nc.sync.dma_start(pwc[:], dr['pw'][:, ch * 1024:(ch + 1) * 1024]
                                  .rearrange("(k p) m -> p k m", p=P))
                for tc2 in range(2):
                    osb = outp.tile([P, 8, 512], F32, name="osb")
                    for no8 in range(8):
                        ps = mmps.tile([P, 512], F32, name="mm")
                        for kk in range(KC):
                            nc.tensor.matmul(ps[:], pwc[:, kk, no8 * P:(no8 + 1) * P],
                                             gT[:, kk, tc2 * 512:(tc2 + 1) * 512],
                                             start=(kk == 0), stop=(kk == KC - 1))
                        no = ch * 8 + no8
                        nc.scalar.activation(osb[:, no8], ps[:], AF.Identity,
                                             bias=pwb_sb[:, no:no + 1])
                    nc.sync.dma_start(
                        outT[ch * 1024:(ch + 1) * 1024, tc2 * 512:(tc2 + 1) * 512]
                        .rearrange("(n p) l -> p n l", p=P), osb[:])

    nc.compile()
    _BUILD_CACHE[key] = nc
    return nc


LAST_RESULTS = None


def kernel(**inputs):
    global LAST_RESULTS
    from concourse import bass_utils
    sh, xts = _prepare(inputs)
    nc = _build()
    in_maps = []
    for c in range(_NCORES):
        m = dict(sh)
        m['xT'] = xts[c % B]
        in_maps.append(m)
    trace = os.environ.get("BT_TRACE", "0") == "1"
    if trace:
        try:
            import antenv.axon_hooks  # noqa: F401
        except ImportError:
            trace = False
    res = bass_utils.run_bass_kernel_spmd(nc, in_maps, core_ids=list(range(_NCORES)),
                                          trace=trace)
    LAST_RESULTS = res
    outs = []
    for c in range(B):
        oT = res.results[c % _NCORES]['outT']  # [N_E, L] in device token order
        o = oT.T[_WM_INV]                      # [L, N_E] raster order
        outs.append(o)
    return np.stack(outs).astype(np.float32)


# revision 24
# speedup vs baseline: 1.0850x; 1.0848x over previous
import os
import numpy as np

# ---- problem constants (hardcoded; kernel.py must be self-contained) ----
IMG, WS, SHIFT = 32, 8, 4
C, HEADS, DEPTH = 512, 16, 24
E_DIM, N_E, B = 256, 8192, 8
L = IMG * IMG            # 1024
NW = WS * WS             # 64 tokens per window
HD = C // HEADS          # 32
NWIN = (IMG // WS) ** 2  # 16
FH = 4 * C               # 2048
P = 128
VBLK = HD + 2            # 34 (32 vals + softmax-denominator col + pad)
VW = HEADS * VBLK        # 544
NT = L // P              # 8 token tiles
KC = C // P              # 4 k-tiles over C
KE = E_DIM // P          # 2 k-tiles over E_DIM
NH = FH // P             # 16 fc1 out-channel tiles
SCALE = HD ** -0.5

_DEPTH = int(os.environ.get("BT_DEPTH", DEPTH))
_NCORES = int(os.environ.get("BT_NCORES", 8))


# ---- host-side helpers (mirror reference.py) ----
def _rel_index():
    coords = np.stack(np.meshgrid(np.arange(WS), np.arange(WS), indexing='ij'))
    cf = coords.reshape(2, -1)
    rel = (cf[:, :, None] - cf[:, None, :]).transpose(1, 2, 0)
    rel[:, :, 0] += WS - 1
    rel[:, :, 1] += WS - 1
    rel[:, :, 0] *= 2 * WS - 1
    return rel.sum(-1)  # [NW, NW] int


def _shift_mask():
    img = np.zeros((IMG, IMG), np.float32)
    cnt = 0
    sl = (slice(0, -WS), slice(-WS, -SHIFT), slice(-SHIFT, None))
    for hs in sl:
        for ws_ in sl:
            img[hs, ws_] = cnt
            cnt += 1
    win = img.reshape(IMG // WS, WS, IMG // WS, WS).transpose(0, 2, 1, 3).reshape(-1, NW)
    diff = win[:, None, :] - win[:, :, None]
    return np.where(diff != 0, -100.0, 0.0).astype(np.float32)  # [NWIN, NW, NW]


def _win_perm():
    """raster token index -> window-major position; perm[t_raster] = t_dev"""
    t = np.arange(L).reshape(IMG, IMG)
    wm = t.reshape(IMG // WS, WS, IMG // WS, WS).transpose(0, 2, 1, 3).reshape(-1)
    inv = np.empty(L, np.int64)
    inv[wm] = np.arange(L)
    return wm, inv  # wm: dev->raster, inv: raster->dev


_WM, _WM_INV = _win_perm()
_REL = _rel_index()
_MASK = _shift_mask()


def _prepare(inputs):
    import ml_dtypes
    bf16 = ml_dtypes.bfloat16
    f32 = lambda a: np.ascontiguousarray(a, dtype=np.float32)
    b16 = lambda a: np.ascontiguousarray(np.asarray(a, np.float32), dtype=bf16)
    x = np.asarray(inputs['x'], np.float32)          # [B, L, E]
    dec_w = np.asarray(inputs['dec_w'], np.float32)  # [C, E]
    dec_b = np.asarray(inputs['dec_b'], np.float32)
    pos = np.asarray(inputs['pos_embed'], np.float32)[0]  # [L, C]
    n1w = np.asarray(inputs['n1w'], np.float32)
    n1b = np.asarray(inputs['n1b'], np.float32)
    qkv_w = np.asarray(inputs['qkv_w'], np.float32)
    qkv_b = np.asarray(inputs['qkv_b'], np.float32)
    proj_w = np.asarray(inputs['proj_w'], np.float32)
    proj_b = np.asarray(inputs['proj_b'], np.float32)
    rel_bias = np.asarray(inputs['rel_bias'], np.float32)
    n2w = np.asarray(inputs['n2w'], np.float32)
    n2b = np.asarray(inputs['n2b'], np.float32)
    fc1_w = np.asarray(inputs['fc1_w'], np.float32)
    fc1_b = np.asarray(inputs['fc1_b'], np.float32)
    fc2_w = np.asarray(inputs['fc2_w'], np.float32)
    fc2_b = np.asarray(inputs['fc2_b'], np.float32)
    normf_w = np.asarray(inputs['normf_w'], np.float32)
    normf_b = np.asarray(inputs['normf_b'], np.float32)
    pred_w = np.asarray(inputs['pred_w'], np.float32)
    pred_b = np.asarray(inputs['pred_b'], np.float32)

    D = _DEPTH
    sh = {}
    sh['decw'] = b16(dec_w.T)                                    # [E, C]
    sh['posb'] = f32((pos + dec_b[None, :])[_WM])                # [L, C]

    wqk = np.empty((D, C, 2 * C), np.float32)
    bqk = np.zeros((D, P, 8), np.float32)
    wvp = np.zeros((D, C, VW), np.float32)
    vb = np.zeros((D, P, VW), np.float32)
    ab = np.empty((D, NT, 4, P, 4, NW), np.float32)  # (wp, rg, 2w*tk, hi, tq)
    wp_ = np.empty((D, C, C), np.float32)
    pb = np.empty((D, P, C), np.float32)
    f1 = np.empty((D, C, FH), np.float32)
    f1b = np.empty((D, P, NH), np.float32)
    f2 = np.empty((D, FH, C), np.float32)
    f2b = np.empty((D, P, C), np.float32)

    for i in range(D):
        Wm = qkv_w[i] * n1w[i][None, :]           # [3C, C]
        bm = qkv_w[i] @ n1b[i] + qkv_b[i]         # [3C]
        Wm = Wm.copy()
        bm = bm.copy()
        Wm[:C] *= SCALE
        bm[:C] *= SCALE
        wqk[i] = Wm[:2 * C].T                     # [C, 2C]
        bqk[i] = bm[:2 * C].reshape(8, P).T       # bias for out-channel tile mo at [:, mo]
        # v with padded 34-blocks; ones column for the softmax denominator
        for h in range(HEADS):
            wvp[i][:, h * VBLK:h * VBLK + HD] = Wm[2 * C + h * HD:2 * C + (h + 1) * HD].T
            vb[i][:, h * VBLK:h * VBLK + HD] = bm[2 * C + h * HD:2 * C + (h + 1) * HD][None, :]
            vb[i][:, h * VBLK + HD] = 1.0
        # attention additive bias [tk, tq] per (win, head)
        bias = rel_bias[i][_REL]                  # [tq, tk, HEADS]
        shift = (i % 2) == 1
        for w in range(NWIN):
            for h in range(HEADS):
                a = bias[:, :, h].T               # [tk, tq]
                if shift:
                    a = a + _MASK[w].T
                wp2, w01 = w // 2, w % 2
                rg, hi = h % 4, h // 4
                ab[i, wp2, rg, w01 * NW:(w01 + 1) * NW, hi, :] = a
        wp_[i] = proj_w[i].T
        pb[i] = np.broadcast_to(proj_b[i][None, :], (P, C))
        f1[i] = (fc1_w[i] * n2w[i][None, :]).T    # [C, FH]
        f1b[i] = (fc1_w[i] @ n2b[i] + fc1_b[i]).reshape(NH, P).T
        f2[i] = fc2_w[i].T
        f2b[i] = np.broadcast_to(fc2_b[i][None, :], (P, C))

    sh['wqk'] = b16(wqk)
    sh['bqk'] = f32(bqk)
    sh['wvp'] = b16(wvp)
    sh['vb'] = f32(vb)
    # ab partition-major for one contiguous DMA per layer: [D, P, NT, 4, 4*NW]
    sh['ab'] = np.ascontiguousarray(
        ab.reshape(D, NT, 4, P, 4 * NW).transpose(0, 3, 1, 2, 4).astype(bf16))
    sh['wp'] = b16(wp_)
    sh['pb'] = f32(pb)
    sh['f1'] = b16(f1)
    sh['f1b'] = f32(f1b)
    sh['f2'] = b16(f2)
    sh['f2b'] = f32(f2b)
    sh['nfw'] = f32(normf_w.reshape(KC, P).T)     # [P, KC]
    sh['nfb'] = f32(normf_b.reshape(KC, P).T)
    sh['pw'] = b16(pred_w.T)                      # [C, N_E]
    sh['pwb'] = f32(pred_b.reshape(N_E // P, P).T)  # [P, 64]
    # per-core xT in device token order: [E, L] bf16
    xts = [np.ascontiguousarray(x[c][_WM].T.astype(bf16)) for c in range(B)]
    return sh, xts


# ---- device program ----
_BUILD_CACHE = {}


def _build():
    key = (_DEPTH,)
    if key in _BUILD_CACHE:
        return _BUILD_CACHE[key]
    import concourse.bass as bass
    import concourse.mybir as mybir
    import concourse.tile as tile
    from concourse import bacc
    from concourse.masks import make_identity
    from contextlib import ExitStack

    F32 = mybir.dt.float32
    BF16 = mybir.dt.bfloat16
    AF = mybir.ActivationFunctionType
    ALU = mybir.AluOpType
    AX = mybir.AxisListType
    D = _DEPTH

    nc = bacc.Bacc("TRN2", target_bir_lowering=False, debug=False, num_devices=_NCORES)

    dr = {}
    def din(name, shape, dt):
        dr[name] = nc.dram_tensor(name, list(shape), dt, kind="ExternalInput").ap()
    din('xT', (E_DIM, L), BF16)
    din('decw', (E_DIM, C), BF16)
    din('posb', (L, C), F32)
    din('wqk', (D, C, 2 * C), BF16)
    din('bqk', (D, P, 8), F32)
    din('wvp', (D, C, VW), BF16)
    din('vb', (D, P, VW), F32)
    din('ab', (D, P, NT, 4, 4 * NW), BF16)
    din('wp', (D, C, C), BF16)
    din('pb', (D, P, C), F32)
    din('f1', (D, C, FH), BF16)
    din('f1b', (D, P, NH), F32)
    din('f2', (D, FH, C), BF16)
    din('f2b', (D, P, C), F32)
    din('nfw', (P, KC), F32)
    din('nfb', (P, KC), F32)
    din('pw', (C, N_E), BF16)
    din('pwb', (P, N_E // P), F32)
    outT = nc.dram_tensor("outT", [N_E, L], F32, kind="ExternalOutput").ap()

    with tile.TileContext(nc) as tc, ExitStack() as ES:
        # ---------- persistent SBUF pools ----------
        cst = ES.enter_context(tc.tile_pool(name="cst", bufs=1))
        ident_f = cst.tile([P, P], F32)
        make_identity(nc, ident_f)
        ident_b = cst.tile([P, P], BF16)
        nc.scalar.copy(ident_b[:], ident_f[:])

        xp = ES.enter_context(tc.tile_pool(name="xp", bufs=1))
        hp = ES.enter_context(tc.tile_pool(name="hp", bufs=1))
        hTp = ES.enter_context(tc.tile_pool(name="hTp", bufs=3))
        qkp = ES.enter_context(tc.tile_pool(name="qkp", bufs=1))
        vp = ES.enter_context(tc.tile_pool(name="vp", bufs=1))
        attp = ES.enter_context(tc.tile_pool(name="attp", bufs=1))
        ppool = ES.enter_context(tc.tile_pool(name="ppool", bufs=2))
        stp = ES.enter_context(tc.tile_pool(name="stp", bufs=2))
        recp = ES.enter_context(tc.tile_pool(name="recp", bufs=2))
        gp = ES.enter_context(tc.tile_pool(name="gp", bufs=3))
        bp = ES.enter_context(tc.tile_pool(name="bp", bufs=2))

        x = xp.tile([P, NT, C], F32)

        # ---------- layer-norm (stats per tile, rsqrt batched per 4-tile group;
        # rsqrt on DVE via bit-trick + 2 Newton iters: no ACT Sqrt table swaps) ----------
        UINT32 = mybir.dt.uint32

        def ln_stats(mvg, tt, j):
            st6 = stp.tile([P, 6], F32, name="st6", tag=f"st6{tt % 2}")
            nc.vector.bn_stats(st6[:], x[:, tt])
            nc.vector.bn_aggr(mvg[:, j], st6[:])

        def emit_ln(dst, tt, tag):
            st6 = stp.tile([P, 6], F32, name="st6", tag=f"st6{tag}{tt % 4}")
            nc.vector.bn_stats(st6[:], x[:, tt])
            mv = stp.tile([P, 2], F32, name="mv", tag=f"mv{tag}{tt % 4}")
            nc.vector.bn_aggr(mv[:], st6[:])
            ve = stp.tile([P, 1], F32, name="ve", tag=f"ve{tag}{tt % 4}")
            nc.vector.tensor_scalar_add(ve[:], mv[:, 1:2], 1e-5)
            rstd = stp.tile([P, 1], F32, name="rstd", tag=f"rstd{tag}{tt % 4}")
            nc.scalar.activation(rstd[:], ve[:], AF.Sqrt)
            nc.vector.reciprocal(rstd[:], rstd[:])
            nb = stp.tile([P, 1], F32, name="nb", tag=f"nb{tag}{tt % 4}")
            nc.vector.tensor_mul(nb[:], mv[:, 0:1], rstd[:])
            nc.vector.tensor_scalar_mul(nb[:], nb[:], -1.0)
            nc.scalar.activation(dst[:, tt], x[:, tt], AF.Identity,
                                 bias=nb[:], scale=rstd[:])

        # transpose token-major [P, NT, C] bf16 -> C-major [P, KC, L] bf16
        # tt-outer; after each odd tile, cb(sa) can emit the shift-permute
        # pieces whose source tokens are now complete (overlaps PE with DVE/ACT)
        def transpose_to(hT, src, tpool, cb=None):
            for tt in range(NT):
                for ct in range(KC):
                    tps = tpool.tile([P, P], BF16, name="tp")
                    nc.tensor.transpose(tps[:], src[:, tt, ct * P:(ct + 1) * P], ident_b[:])
                    if (ct + tt) % 2 == 0:
                        nc.scalar.copy(hT[:, ct, tt * P:(tt + 1) * P], tps[:])
                    else:
                        nc.vector.tensor_copy(hT[:, ct, tt * P:(tt + 1) * P], tps[:])
                if cb is not None and tt % 2 == 1:
                    cb(tt // 2)

        # shift permute between A-order (unshifted window-major) and B-order.
        # Emits only the pieces reading source row-block `src_a` (= token tiles
        # 2*src_a, 2*src_a+1), spread across gpsimd/vector/scalar.
        def permute_pieces(dstT, srcT, fwd, src_a):
            G = IMG // WS  # 4
            sv = srcT[:].rearrange("p k (a b i j) -> p k a b i j", a=G, b=G, i=WS, j=WS)
            dv = dstT[:].rearrange("p k (a b i j) -> p k a b i j", a=G, b=G, i=WS, j=WS)
            engs = (nc.gpsimd.tensor_copy, nc.vector.tensor_copy, nc.scalar.copy)
            idx = src_a
            for qa in range(2):
                for qb in range(2):
                    di = slice(0, 4) if qa == 0 else slice(4, 8)
                    si = slice(4, 8) if qa == 0 else slice(0, 4)
                    dj = slice(0, 4) if qb == 0 else slice(4, 8)
                    sj = slice(4, 8) if qb == 0 else slice(0, 4)
                    for a in range(G):
                        sa = (a + qa) % G
                        if (sa if fwd else a) != src_a:
                            continue
                        if qb == 0:
                            bpairs = [(slice(0, G), slice(0, G))]
                        else:
                            bpairs = [(slice(0, G - 1), slice(1, G)), (slice(G - 1, G), slice(0, 1))]
                        for db, sb_ in bpairs:
                            for ct in range(KC):
                                eng = engs[idx % 3]
                                idx += 1
                                if fwd:
                                    eng(dv[:, ct, a, db, di, dj],
                                        sv[:, ct, sa, sb_, si, sj])
                                else:
                                    eng(dv[:, ct, sa, sb_, si, sj],
                                        sv[:, ct, a, db, di, dj])

        # ---------- dec ----------
        with tc.tile_pool(name="decp", bufs=1) as decp, \
             tc.tile_pool(name="dps", bufs=2, space="PSUM") as dps:
            xT_sb = decp.tile([P, KE, L], BF16)
            nc.sync.dma_start(xT_sb[:], dr['xT'].rearrange("(k p) t -> p k t", p=P))
            decw_sb = decp.tile([P, KE, C], BF16)
            nc.sync.dma_start(decw_sb[:], dr['decw'].rearrange("(k p) c -> p k c", p=P))
            for tt in range(NT):
                pos_t = decp.tile([P, C], F32, name="pos_t", tag="pos", bufs=2)
                nc.sync.dma_start(pos_t[:], dr['posb'][tt * P:(tt + 1) * P, :])
                ps = dps.tile([P, C], F32)
                for kk in range(KE):
                    nc.tensor.matmul(ps[:], xT_sb[:, kk, tt * P:(tt + 1) * P],
                                     decw_sb[:, kk, :], start=(kk == 0), stop=(kk == KE - 1))
                nc.vector.tensor_add(x[:, tt], ps[:], pos_t[:])

        # LN1 + hT for layer 0 (layer 0 is unshifted)
        h = hp.tile([P, NT, C], BF16, name="h")
        for tt in range(NT):
            emit_ln(h, tt, "a")
        with tc.tile_pool(name="tp0", bufs=4, space="PSUM") as tpool:
            hT = hTp.tile([P, KC, L], BF16, name="hT")
            transpose_to(hT, h, tpool)

        # ---------- weight pools (freed before the pred phase) ----------
        with tc.tile_pool(name="wqkp", bufs=2) as wqkp, \
             tc.tile_pool(name="wvpp", bufs=2) as wvpp, \
             tc.tile_pool(name="wpp", bufs=2) as wpp, \
             tc.tile_pool(name="f1p", bufs=1) as f1p, \
             tc.tile_pool(name="f2p", bufs=1) as f2p, \
             tc.tile_pool(name="abp", bufs=1) as abp:

            # ---------- layers ----------
            for i in range(D):
                shift = (i % 2) == 1
                shift_next = ((i + 1) % 2) == 1
                # whole-layer weight DMAs (one dispatch per tensor)
                wqk_sb = wqkp.tile([P, KC, 2 * C], BF16, name="wqk")
                nc.sync.dma_start(wqk_sb[:], dr['wqk'][i].rearrange("(k p) m -> p k m", p=P))
                bqk_sb = bp.tile([P, 8], F32, name="bqk", tag="bqk")
                nc.sync.dma_start(bqk_sb[:], dr['bqk'][i])
                wvp_sb = wvpp.tile([P, KC, VW], BF16, name="wvp")
                nc.sync.dma_start(wvp_sb[:], dr['wvp'][i].rearrange("(k p) m -> p k m", p=P))
                vb_sb = bp.tile([P, VW], F32, name="vb", tag="vb")
                nc.sync.dma_start(vb_sb[:], dr['vb'][i])
                ab_sb = abp.tile([P, NT, 4, 4 * NW], BF16, name="ab")
                nc.sync.dma_start(ab_sb[:], dr['ab'][i])
                wp_sb = wpp.tile([P, KC, C], BF16, name="wp")
                nc.sync.dma_start(wp_sb[:], dr['wp'][i].rearrange("(k p) m -> p k m", p=P))
                pb_sb = bp.tile([P, C], F32, name="pb", tag="pb")
                nc.sync.dma_start(pb_sb[:], dr['pb'][i])
                f1_sb = f1p.tile([P, KC, FH], BF16, name="f1")
                nc.sync.dma_start(f1_sb[:], dr['f1'][i].rearrange("(k p) m -> p k m", p=P))
                f1b_sb = bp.tile([P, NH], F32, name="f1b", tag="f1b")
                nc.sync.dma_start(f1b_sb[:], dr['f1b'][i])
                f2_sb = f2p.tile([P, NH, C], BF16, name="f2")
                nc.sync.dma_start(f2_sb[:], dr['f2'][i].rearrange("(h p) c -> p h c", p=P))
                f2b_sb = bp.tile([P, C], F32, name="f2b", tag="f2b")
                nc.sync.dma_start(f2b_sb[:], dr['f2b'][i])

                # ---- qk ---- (hT was prepared, incl. fwd shift-permute, at the
                # end of the previous layer)
                qkT = qkp.tile([P, 8, L], BF16, name="qkT")
                with tc.tile_pool(name="mmps1", bufs=2, space="PSUM") as mmps, \
                     tc.tile_pool(name="vps", bufs=2, space="PSUM") as vps:
                    for mo in range(8):
                        for tc2 in range(2):
                            ps = mmps.tile([P, C], F32, name="mm")
                            for kk in range(KC):
                                nc.tensor.matmul(ps[:], wqk_sb[:, kk, mo * P:(mo + 1) * P],
                                                 hT[:, kk, tc2 * 512:(tc2 + 1) * 512],
                                                 start=(kk == 0), stop=(kk == KC - 1))
                            nc.vector.tensor_scalar_add(
                                qkT[:, mo, tc2 * 512:(tc2 + 1) * 512], ps[:],
                                bqk_sb[:, mo:mo + 1])
                    # ---- v ---- (pairs of tiles: all N=512 MMs first, then the
                    # N=32 tails back-to-back so their LDWEIGHTS don't collide
                    # with a 512-wide stream)
                    v_aug = vp.tile([P, NT, VW], BF16, name="vaug")
                    for tp2_ in range(NT // 2):
                        pspair = []
                        for half in range(2):
                            tt = tp2_ * 2 + half
                            ps = vps.tile([P, VW], F32, name="vps")
                            pspair.append(ps)
                            for kk in range(KC):
                                nc.tensor.matmul(ps[:, 0:512], hT[:, kk, tt * P:(tt + 1) * P],
                                                 wvp_sb[:, kk, 0:512], start=(kk == 0),
                                                 stop=(kk == KC - 1), skip_group_check=True)
                        for half in range(2):
                            tt = tp2_ * 2 + half
                            ps = pspair[half]
                            for kk in range(KC):
                                nc.tensor.matmul(ps[:, 512:VW], hT[:, kk, tt * P:(tt + 1) * P],
                                                 wvp_sb[:, kk, 512:VW], start=(kk == 0),
                                                 stop=(kk == KC - 1), skip_group_check=True)
                            nc.vector.tensor_add(v_aug[:, tt], ps[:], vb_sb[:])

                # pre-add proj bias into the residual (gpsimd; overlaps attention)
                for tt in range(NT):
                    nc.gpsimd.tensor_add(x[:, tt], x[:, tt], pb_sb[:])

                # ---- attention: S + softmax + AV + transpose-back, all
                # software-pipelined so the PE never waits on exp/recip/mul
                # latencies (S of wp2+1 before AV of wp2; attT of wp2-2 after
                # AV of wp2-1; back shift-permute pieces woven in) ----
                att = attp.tile([P, NT, C], BF16, name="att")
                aT_B = hTp.tile([P, KC, L], BF16, name="hT")
                if shift:
                    aT = hTp.tile([P, KC, L], BF16, name="hT")
                else:
                    aT = aT_B
                with tc.tile_pool(name="sps", bufs=1, space="PSUM") as sps, \
                     tc.tile_pool(name="avps", bufs=1, space="PSUM") as avps, \
                     tc.tile_pool(name="tp2", bufs=2, space="PSUM") as tpool:
                    def emit_S(wp2):
                        # two rg share one PSUM bank; all MMs first, exps after
                        # (so PE never waits on an exp reading a shared bank)
                        sp2s = []
                        for rp in range(2):
                            sp2 = sps.tile([P, 2, 4, NW], F32, name=f"s{rp}", tag=f"s{rp}")
                            sp2s.append(sp2)
                            for r2 in range(2):
                                rg = rp * 2 + r2
                                nc.tensor.matmul(sp2[:, r2].rearrange("p a b -> p (a b)"),
                                                 ident_b[:], ab_sb[:, wp2, rg],
                                                 start=True, stop=False, skip_group_check=True)
                                for hi in range(4):
                                    for w01 in range(2):
                                        qs = qkT[rg * HD:(rg + 1) * HD, hi,
                                                 (wp2 * 2 + w01) * NW:(wp2 * 2 + w01 + 1) * NW]
                                        ks = qkT[rg * HD:(rg + 1) * HD, 4 + hi,
                                                 (wp2 * 2 + w01) * NW:(wp2 * 2 + w01 + 1) * NW]
                                        nc.tensor.matmul(
                                            sp2[w01 * NW:(w01 + 1) * NW, r2, hi, :], ks, qs,
                                            start=False, stop=(hi == 3 and w01 == 1),
                                            tile_position=(rg * HD, w01 * NW),
                                            skip_group_check=True)
                        pts = []
                        for rg in range(4):
                            pt = ppool.tile([P, 4, NW], BF16, name=f"p{rg}", tag=f"p{rg}")
                            nc.scalar.activation(pt[:].rearrange("p a b -> p (a b)"),
                                                 sp2s[rg // 2][:, rg % 2].rearrange("p a b -> p (a b)"),
                                                 AF.Exp)
                            pts.append(pt)
                        return pts

                    def emit_AV(wp2, pts):
                        for w01 in range(2):
                            rows = slice(w01 * NW, (w01 + 1) * NW)
                            rec = recp.tile([P, HEADS], F32, name=f"rec{w01}", tag=f"rec{w01}")
                            for half in range(2):
                                av = avps.tile([P, 8, VBLK], F32, name=f"av{w01}{half}", tag=f"av{w01}{half}")
                                for hh in range(8):
                                    hglob = half * 8 + hh
                                    hi, rg = hglob // 4, hglob % 4
                                    nc.tensor.matmul(
                                        av[rows, hh, :], pts[rg][rows, hi, :],
                                        v_aug[rows, wp2, hglob * VBLK:(hglob + 1) * VBLK],
                                        start=True, stop=True,
                                        tile_position=(w01 * NW, w01 * NW))
                                nc.vector.reciprocal(rec[rows, half * 8:(half + 1) * 8],
                                                     av[rows, :, HD])
                                rb = rec[rows, half * 8:(half + 1) * 8] \
                                    .rearrange("p (a b) -> p a b", b=1).to_broadcast((NW, 8, HD))
                                dst = att[rows, wp2, half * 256:(half + 1) * 256] \
                                    .rearrange("p (a b) -> p a b", b=HD)
                                nc.vector.tensor_mul(dst, av[rows, :, 0:HD], rb)

                    def emit_attT(tt):
                        for ct in range(KC):
                            tps = tpool.tile([P, P], BF16, name="tp")
                            nc.tensor.transpose(tps[:], att[:, tt, ct * P:(ct + 1) * P], ident_b[:])
                            if (ct + tt) % 2 == 0:
                                nc.scalar.copy(aT_B[:, ct, tt * P:(tt + 1) * P], tps[:])
                            else:
                                nc.vector.tensor_copy(aT_B[:, ct, tt * P:(tt + 1) * P], tps[:])
                        if shift and tt % 2 == 1:
                            permute_pieces(aT, aT_B, False, tt // 2)

                    ptsd = {0: emit_S(0)}
                    ptsd[1] = emit_S(1)
                    emit_AV(0, ptsd.pop(0))
                    for wp2 in range(2, NT):
                        ptsd[wp2] = emit_S(wp2)
                        emit_AV(wp2 - 1, ptsd.pop(wp2 - 1))
                        emit_attT(wp2 - 2)
                    emit_AV(NT - 1, ptsd.pop(NT - 1))
                    emit_attT(NT - 2)
                    emit_attT(NT - 1)

                # ---- proj + residual + LN2 per tile, h2T transposes delayed
                # by two tiles so they never gate the PE on the LN chain ----
                h2 = hp.tile([P, NT, C], BF16, name="h")
                h2T = hTp.tile([P, KC, L], BF16, name="hT")
                with tc.tile_pool(name="mmps2", bufs=2, space="PSUM") as mmps, \
                     tc.tile_pool(name="tp3", bufs=2, space="PSUM") as tpool:
                    def emit_h2T(tt):
                        for ct in range(KC):
                            tps = tpool.tile([P, P], BF16, name="tp")
                            nc.tensor.transpose(tps[:], h2[:, tt, ct * P:(ct + 1) * P], ident_b[:])
                            if (ct + tt) % 2 == 0:
                                nc.scalar.copy(h2T[:, ct, tt * P:(tt + 1) * P], tps[:])
                            else:
                                nc.vector.tensor_copy(h2T[:, ct, tt * P:(tt + 1) * P], tps[:])
                    for tt in range(NT):
                        ps = mmps.tile([P, C], F32, name="mm")
                        for kk in range(KC):
                            nc.tensor.matmul(ps[:], aT[:, kk, tt * P:(tt + 1) * P],
                                             wp_sb[:, kk, :], start=(kk == 0), stop=(kk == KC - 1))
                        nc.vector.tensor_add(x[:, tt], ps[:], x[:, tt])
                        emit_ln(h2, tt, "b")
                        # pre-add fc2 bias (gpsimd; after LN2 read x)
                        nc.gpsimd.tensor_add(x[:, tt], x[:, tt], f2b_sb[:])
                        if tt >= 2:
                            emit_h2T(tt - 2)
                    emit_h2T(NT - 2)
                    emit_h2T(NT - 1)

                # ---- MLP (epilogue emits LN1 of next layer / final LN); the
                # next layer's hT transposes for tiles 0-3 are woven into the
                # tc2=1 stream so the PE stays warm across the layer boundary ----
                h = hp.tile([P, NT, C], BF16, name="h")
                if i < D - 1:
                    hT_A = hTp.tile([P, KC, L], BF16, name="hT")
                    if shift_next:
                        hTn = hTp.tile([P, KC, L], BF16, name="hT")
                        hT = hTn
                    else:
                        hT = hT_A
                with tc.tile_pool(name="mmps3", bufs=2, space="PSUM") as mmps, \
                     tc.tile_pool(name="fc2ps", bufs=1, space="PSUM") as fc2ps, \
                     tc.tile_pool(name="tp4", bufs=2, space="PSUM") as tpool:
                    def emit_hT(tt):
                        for ct in range(KC):
                            tps = tpool.tile([P, P], BF16, name="tp")
                            nc.tensor.transpose(tps[:], h[:, tt, ct * P:(ct + 1) * P], ident_b[:])
                            if (ct + tt) % 2 == 0:
                                nc.scalar.copy(hT_A[:, ct, tt * P:(tt + 1) * P], tps[:])
                            else:
                                nc.vector.tensor_copy(hT_A[:, ct, tt * P:(tt + 1) * P], tps[:])
                        if shift_next and tt % 2 == 1:
                            permute_pieces(hTn, hT_A, True, tt // 2)
                    for tc2 in range(2):
                        pso = [fc2ps.tile([P, C], F32, name=f"fc2_{j}", tag=f"fc2_{j}") for j in range(4)]

                        def emit_fc2(ho, g, last):
                            for j in range(4):
                                nc.tensor.matmul(pso[j][:], g[:, j * P:(j + 1) * P],
                                                 f2_sb[:, ho, :],
                                                 start=(ho == 0), stop=last)
                        g_prev = None
                        for ho in range(NH):
                            ps1 = mmps.tile([P, C], F32, name="mm")
                            for kk in range(KC):
                                nc.tensor.matmul(ps1[:], f1_sb[:, kk, ho * P:(ho + 1) * P],
                                                 h2T[:, kk, tc2 * 512:(tc2 + 1) * 512],
                                                 start=(kk == 0), stop=(kk == KC - 1))
                            g = gp.tile([P, C], BF16, name="g")
                            nc.scalar.activation(g[:], ps1[:], AF.Gelu, bias=f1b_sb[:, ho:ho + 1])
                            if g_prev is not None:
                                emit_fc2(ho - 1, g_prev, False)
                            g_prev = g
                            if tc2 == 1 and i < D - 1 and 6 <= ho <= 9:
                                emit_hT(ho - 6)
                        emit_fc2(NH - 1, g_prev, True)
                        for j in range(4):
                            tt = tc2 * 4 + j
                            nc.vector.tensor_add(x[:, tt], pso[j][:], x[:, tt])
                            emit_ln(h, tt, "a")
                    if i < D - 1:
                        for tt in range(4, NT):
                            emit_hT(tt)
                    # final layer: h holds plain-LN output for the pred head

        # ---------- final gelu(LN)*nfw+nfb transpose + pred ----------
        nfw_sb = bp.tile([P, KC], F32, name="nfw", tag="nfw")
        nc.sync.dma_start(nfw_sb[:], dr['nfw'])
        nfb_sb = bp.tile([P, KC], F32, name="nfb", tag="nfb")
        nc.sync.dma_start(nfb_sb[:], dr['nfb'])
        pwb_sb = bp.tile([P, N_E // P], F32, name="pwb", tag="pwb", bufs=1)
        nc.sync.dma_start(pwb_sb[:], dr['pwb'])
        with tc.tile_pool(name="tpf", bufs=4, space="PSUM") as tpool, \
             tc.tile_pool(name="mmpsf", bufs=4, space="PSUM") as mmps, \
             tc.tile_pool(name="pwp", bufs=2) as pwp, \
             tc.tile_pool(name="outp", bufs=2) as outp:
            gT = hTp.tile([P, KC, L], BF16, name="hT")
            for ct in range(KC):
                for tt in range(NT):
                    tps = tpool.tile([P, P], BF16, name="tp")
                    nc.tensor.transpose(tps[:], h[:, tt, ct * P:(ct + 1) * P], ident_b[:])
                    nc.scalar.activation(gT[:, ct, tt * P:(tt + 1) * P], tps[:], AF.Gelu,
                                         bias=nfb_sb[:, ct:ct + 1], scale=nfw_sb[:, ct:ct + 1])
            # pred: out C-major [N_E, L]; 8 weight chunks, big batched output DMAs
            for ch in range(8):
                pwc = pwp.tile([P, KC, 8 * P], BF16, name="pwc")
                nc.sync.dma_start(pwc[:], dr['pw'][:, ch * 1024:(ch + 1) * 1024]
                                  .rearrange("(k p) m -> p k m", p=P))
                for tc2 in range(2):
                    osb = outp.tile([P, 8, 512], F32, name="osb")
                    for no8 in range(8):
                        ps = mmps.tile([P, 512], F32, name="mm")
                        for kk in range(KC):
                            nc.tensor.matmul(ps[:], pwc[:, kk, no8 * P:(no8 + 1) * P],
                                             gT[:, kk, tc2 * 512:(tc2 + 1) * 512],
                                             start=(kk == 0), stop=(kk == KC - 1))
                        no = ch * 8 + no8
                        nc.scalar.activation(osb[:, no8], ps[:], AF.Identity,
                                             bias=pwb_sb[:, no:no + 1])
                    nc.sync.dma_start(
                        outT[ch * 1024:(ch + 1) * 1024, tc2 * 512:(tc2 + 1) * 512]
                        .rearrange("(n p) l -> p n l", p=P), osb[:])

    nc.compile()
    _BUILD_CACHE[key] = nc
    return nc


LAST_RESULTS = None


def kernel(**inputs):
    global LAST_RESULTS
    from concourse import bass_utils
    sh, xts = _prepare(inputs)
    nc = _build()
    in_maps = []
    for c in range(_NCORES):
        m = dict(sh)
        m['xT'] = xts[c % B]
        in_maps.append(m)
    trace = os.environ.get("BT_TRACE", "0") == "1"
    if trace:
        try:
            import antenv.axon_hooks  # noqa: F401
        except ImportError:
            trace = False
    res = bass_utils.run_bass_kernel_spmd(nc, in_maps, core_ids=list(range(_NCORES)),
                                          trace=trace)
    LAST_RESULTS = res
    outs = []
    for c in range(B):
        oT = res.results[c % _NCORES]['outT']  # [N_E, L] in device token order
        o = oT.T[_WM_INV]                      # [L, N_E] raster order
        outs.append(o)
    return np.stack(outs).astype(np.float32)
